# revision 1
# baseline (speedup 1.0000x reference)
"""AnchorFreeLoss on 8 TRN2 NeuronCores.

Strategy (data-parallel over batch, 2 images/core):
- Heatmap target: logG[pix, m] = -dist^2/(2*sigma_m^2) is an affine function of
  q(pix) = [x^2+y^2, x, y, 1]  ->  one PE matmul per 128-pixel chunk with
  per-object coefficient matrix W (built on device from bboxes).
  Output lands in PSUM [128 pix, (2 img, 64 m)]; DVE tensor_reduce(max) along
  the free m axis gives log-heatmap per pixel. Since sigma = r/2 exactly, the
  reference's dist<=2r cutoff equals logG >= -8 (constant!), applied post-max.
- Box/class losses only receive nonzero contributions at object-center cells,
  so pred_boxes/pred_classes are gathered sparsely (indirect DMA over
  host-transposed [B*H*W, C] tables); 77MB of dense reads avoided.
- Duplicate-cell collisions are deduplicated on device with a [128,128]
  is_equal matrix (last valid object wins, matching XLA scatter semantics).
- Partial sums are AllGathered (cheaper than AllReduce) and reduced on-device.
"""

import sys
from contextlib import ExitStack

import numpy as np

if "/opt/trn_rl_repo" not in sys.path:
    sys.path.insert(0, "/opt/trn_rl_repo")

from concourse import bass, bass_isa, mybir
from concourse.bass_utils import run_bass_kernel_spmd

F32 = mybir.dt.float32
I32 = mybir.dt.int32
ALU = mybir.AluOpType
ACT = mybir.ActivationFunctionType

B, M, H, W = 16, 64, 160, 160
NC = 8
BPC = B // NC          # 2 images per core
PIX = H * W            # 25600
NP2 = 2 * M            # 128 = objects of both images on one partition axis
NCLS = 43
EPS = 1e-7
LN4 = 1.3862943611198906
THR = -8.0             # log-domain cutoff (= dist <= 2r since sigma = r/2)
NGRP = PIX // 512      # 50 groups of 4 chunks x 128 pixels
NBANK = 7              # psum pipeline depth


class _DrainAfter:
    """Wrap an engine so every compute op is followed by a pipeline drain().

    Raw-bass DVE/Act programs have no hazard interlock between dependent
    back-to-back ops on the same engine; the interpreter's race detector
    confirms a drain (or semaphore) is required before a read-after-write.
    """

    def __init__(self, eng):
        self._e = eng

    def __getattr__(self, name):
        f = getattr(self._e, name)
        if name in ("wait_ge", "sem_inc", "drain", "then_inc"):
            return f
        def g(*a, **k):
            r = f(*a, **k)
            self._e.drain()
            return r
        return g


def _build(debug=False):
    nc = bass.Bass()

    hm_d = nc.declare_dram_parameter("hm", [BPC, PIX], F32, isOutput=False)
    pb_d = nc.declare_dram_parameter("pbt", [BPC * PIX, 4], F32, isOutput=False)
    pc_d = nc.declare_dram_parameter("pct", [BPC * PIX, NCLS], F32, isOutput=False)
    bb_d = nc.declare_dram_parameter("bb", [BPC, M, 4], F32, isOutput=False)
    lab_d = nc.declare_dram_parameter("lab", [BPC, M], I32, isOutput=False)
    qg_d = nc.declare_dram_parameter("qgrid", [4, PIX], F32, isOutput=False)
    ut_d = nc.declare_dram_parameter("utri", [128, 128], F32, isOutput=False)
    cv_d = nc.declare_dram_parameter("cvec", [128, 8], F32, isOutput=False)
    ch_d = nc.declare_dram_parameter("chm", [128, NCLS], F32, isOutput=False)
    out_d = nc.declare_dram_parameter("out", [1, 1], F32, isOutput=True)
    dbg = {}
    if debug:
        for nm, shp, dt in [("d_partials", [128, 8], F32), ("d_pvec", [1, 8], F32),
                            ("d_agv", [8, 8], F32), ("d_sc", [128, 40], F32)]:
            dbg[nm] = nc.declare_dram_parameter(nm, shp, dt, isOutput=True)

    cc_in = nc.dram_tensor("cc_in", [1, 8], F32)
    cc_out = nc.dram_tensor("cc_out", [8, 8], F32, addr_space="Shared")
    dbc = nc.dram_tensor("dbc", [2, 128], F32)

    core_ids = list(range(NC))

    es = ExitStack()
    dma_in = es.enter_context(nc.semaphore("dma_in"))
    dma2 = es.enter_context(nc.semaphore("dma2"))
    d4 = es.enter_context(nc.semaphore("d4"))
    d5 = es.enter_context(nc.semaphore("d5"))
    d6 = es.enter_context(nc.semaphore("d6"))
    va = es.enter_context(nc.semaphore("va"))
    av = es.enter_context(nc.semaphore("av"))
    wrdy = es.enter_context(nc.semaphore("wrdy"))
    tr_s = es.enter_context(nc.semaphore("tr_s"))
    pe_s = es.enter_context(nc.semaphore("pe_s"))
    dv_s = es.enter_context(nc.semaphore("dv_s"))
    cell_s = es.enter_context(nc.semaphore("cell_s"))
    g_s = es.enter_context(nc.semaphore("g_s"))
    pt_s = es.enter_context(nc.semaphore("pt_s"))
    pp_s = es.enter_context(nc.semaphore("pp_s"))
    pv_s = es.enter_context(nc.semaphore("pv_s"))
    cc_s = es.enter_context(nc.semaphore("cc_s"))
    par_s = es.enter_context(nc.semaphore("par_s"))
    fin_s = es.enter_context(nc.semaphore("fin_s"))
    sQ = es.enter_context(nc.sbuf_tensor("sQ", [4, PIX], F32))
    sU = es.enter_context(nc.sbuf_tensor("sU", [128, 128], F32))
    cvec = es.enter_context(nc.sbuf_tensor("cvec_s", [128, 8], F32))
    chm = es.enter_context(nc.sbuf_tensor("chm_s", [128, NCLS], F32))
    sbb = es.enter_context(nc.sbuf_tensor("sbb", [128, 4], F32))
    slab = es.enter_context(nc.sbuf_tensor("slab", [128, 1], I32))
    hmP = es.enter_context(nc.sbuf_tensor("hmP", [128, 400], F32))
    hmL = es.enter_context(nc.sbuf_tensor("hmL", [128, 400], F32))
    W5 = es.enter_context(nc.sbuf_tensor("W5", [128, 32], F32))
    tmp = es.enter_context(nc.sbuf_tensor("tmp", [32, 128], F32))
    cbc = es.enter_context(nc.sbuf_tensor("cbc", [128, 128], F32))
    kbc = es.enter_context(nc.sbuf_tensor("kbc", [128, 128], F32))
    eqt = es.enter_context(nc.sbuf_tensor("eqt", [128, 128], F32))
    junkm = es.enter_context(nc.sbuf_tensor("junkm", [128, 128], F32))
    partials = es.enter_context(nc.sbuf_tensor("partials", [128, 8], F32))
    gb = es.enter_context(nc.sbuf_tensor("gb", [128, 4], F32))
    gc = es.enter_context(nc.sbuf_tensor("gc", [128, NCLS], F32))
    junk43 = es.enter_context(nc.sbuf_tensor("junk43", [128, NCLS], F32))
    junk4 = es.enter_context(nc.sbuf_tensor("junk4", [128, 4], F32))
    agv = es.enter_context(nc.sbuf_tensor("agv", [8, 8], F32))
    agr = es.enter_context(nc.sbuf_tensor("agr", [8, 8], F32))
    pvec = es.enter_context(nc.sbuf_tensor("pvec", [1, 8], F32))
    res = es.enter_context(nc.sbuf_tensor("res", [1, 1], F32))
    sc = es.enter_context(nc.sbuf_tensor("sc", [128, 40], F32))
    sci = es.enter_context(nc.sbuf_tensor("sci", [128, 4], I32))
    tbox = es.enter_context(nc.sbuf_tensor("tbox", [128, 4], F32))
    gcp = es.enter_context(nc.sbuf_tensor("gcp", [128, NCLS], F32))
    fw0 = es.enter_context(nc.sbuf_tensor("fw0", [128, 400], F32))
    fw1 = es.enter_context(nc.sbuf_tensor("fw1", [128, 400], F32))
    fw2 = es.enter_context(nc.sbuf_tensor("fw2", [128, 400], F32))
    fw3 = es.enter_context(nc.sbuf_tensor("fw3", [128, 400], F32))
    fw4 = es.enter_context(nc.sbuf_tensor("fw4", [128, 400], F32))
    fw5 = es.enter_context(nc.sbuf_tensor("fw5", [128, 400], F32))
    fw6 = es.enter_context(nc.sbuf_tensor("fw6", [128, 400], F32))
    ps0 = es.enter_context(nc.psum_tensor("ps0", [128, 512], F32))
    ps1 = es.enter_context(nc.psum_tensor("ps1", [128, 512], F32))
    ps2 = es.enter_context(nc.psum_tensor("ps2", [128, 512], F32))
    ps3 = es.enter_context(nc.psum_tensor("ps3", [128, 512], F32))
    ps4 = es.enter_context(nc.psum_tensor("ps4", [128, 512], F32))
    ps5 = es.enter_context(nc.psum_tensor("ps5", [128, 512], F32))
    ps6 = es.enter_context(nc.psum_tensor("ps6", [128, 512], F32))
    psp = es.enter_context(nc.psum_tensor("psp", [1, 8], F32))
    with es:
        ps = [ps0, ps1, ps2, ps3, ps4, ps5, ps6]

        # named [128,1] f32 scratch columns
        names = [
            "sumx", "csx", "sumy", "csy", "gxf", "gyf", "bw", "bh", "area",
            "rq", "rm", "rf", "r2", "rr", "nsc", "labf", "vlab", "vbw", "vbh",
            "bbs", "vbs", "valid", "gx2", "gy2", "g2s", "w3a", "pen", "labcf",
            "cellf", "cellgf", "keyf", "cva", "sent", "later1", "kept",
            "later2", "keep2", "l1r", "negrow", "plab",
        ]
        col = {n: sc[:, i : i + 1] for i, n in enumerate(names)}
        idxp1 = cvec[:, 0:1]
        nc.const_aps.aps[(F32, 0.0)] = cvec[:, 3:4]
        nc.const_aps.aps[(F32, 1.0)] = cvec[:, 1:2]
        nc.const_aps.aps[(F32, 1e-6)] = cvec[:, 4:5]
        nc.const_aps.aps[(F32, -LN4)] = cvec[:, 5:6]
        ones = cvec[:, 1:2]
        basef = cvec[:, 2:3]
        gxi = sci[:, 0:1]
        gyi = sci[:, 1:2]
        cellg = sci[:, 2:3]

        with nc.Block() as block:

            @block.sync
            def _(sync):
                sync.dma_start(out=sbb[:, :], in_=bb_d[:, :, :].rearrange("a m c -> (a m) c")).then_inc(dma_in, 16)
                sync.dma_start(out=slab[:, :], in_=lab_d[:, :].rearrange("a m -> (a m)").unsqueeze(1)).then_inc(dma_in, 16)
                sync.dma_start(out=sQ[:, :], in_=qg_d[:, :]).then_inc(dma_in, 16)
                sync.dma_start(out=sU[:, :], in_=ut_d[:, :]).then_inc(dma_in, 16)
                sync.dma_start(out=cvec[:, :], in_=cv_d[:, :]).then_inc(dma_in, 16)
                sync.dma_start(out=chm[:, :], in_=ch_d[:, :]).then_inc(dma_in, 16)
                # hmP[p, 2*fo+img] = hm[img, 128*fo + p]
                sync.dma_start(
                    out=hmP[:, :].rearrange("p (i f) -> p i f", i=2),
                    in_=hm_d[:, :].rearrange("i (p f) -> p i f", p=128),
                ).then_inc(dma_in, 16)
                # broadcast rows 4/5 of tmp via dram bounce
                sync.wait_ge(tr_s, 1)
                sync.dma_start(out=dbc[0:1, :], in_=tmp[4:5, :]).then_inc(dma2, 16)
                sync.dma_start(out=dbc[1:2, :], in_=tmp[5:6, :]).then_inc(dma2, 16)
                sync.wait_ge(dma2, 32)
                sync.dma_start(out=cbc[:, :], in_=dbc[0:1, :].to_broadcast([128, 128])).then_inc(dma2, 16)
                sync.dma_start(out=kbc[:, :], in_=dbc[1:2, :].to_broadcast([128, 128])).then_inc(dma2, 16)
                # partial-sum vector out
                sync.wait_ge(pv_s, 1)
                sync.dma_start(out=cc_in[:, :], in_=pvec[:, :]).then_inc(d4, 16)
                # final result out
                sync.wait_ge(fin_s, 1)
                sync.dma_start(out=out_d[:, :], in_=res[:, :]).then_inc(d6, 16)
                nd6 = 16
                if debug:
                    for nm, t in [("d_partials", partials), ("d_pvec", pvec), ("d_agv", agv),
                                  ("d_sc", sc)]:
                        sync.dma_start(out=dbg[nm][:, :], in_=t[:, :]).then_inc(d6, 16)
                        nd6 += 16
                sync.wait_ge(d6, nd6)

            @block.scalar
            def _(scalar):
                scalar = _DrainAfter(scalar)
                # sqrt(area)/4 = exp(0.5*ln(area) - ln4)
                scalar.wait_ge(va, 1)
                scalar.activation(col["rq"], col["area"], ACT.Ln)
                scalar.activation(col["rq"], col["rq"], ACT.Exp, bias=-LN4, scale=0.5)
                # dw/dh = ln(bw*0.25 + 1e-6)
                scalar.activation(tbox[:, 2:3], col["bw"], ACT.Ln, bias=1e-6, scale=0.25)
                scalar.activation(tbox[:, 3:4], col["bh"], ACT.Ln, bias=1e-6, scale=0.25)
                scalar.sem_inc(av, 1)
                # cls sigmoid via exp
                scalar.wait_ge(g_s, 32)
                scalar.activation(gcp[:, :], gc[:, :], ACT.Exp, scale=-1.0)
                scalar.sem_inc(av, 1)
                # ln(1-p), p^2 for cls neg terms (gcp holds clipped p by now)
                scalar.wait_ge(va, 2)
                scalar.activation(junk43[:, :], gcp[:, :], ACT.Ln, bias=1.0, scale=-1.0)
                scalar.activation(gc[:, :], gcp[:, :], ACT.Square)
                scalar.sem_inc(av, 1)
                # pos-correction transcendentals on plab
                scalar.wait_ge(va, 3)
                scalar.activation(col["sumx"], col["plab"], ACT.Ln)                      # ln p
                scalar.activation(col["sumy"], col["plab"], ACT.Ln, bias=1.0, scale=-1.0)  # ln(1-p)
                scalar.activation(col["gx2"], col["plab"], ACT.Square)                    # p^2
                scalar.activation(col["gy2"], col["plab"], ACT.Square, bias=1.0, scale=-1.0)  # (1-p)^2
                scalar.sem_inc(av, 1)
                # heat: exp of clamped log-heatmap; focal transcendentals
                scalar.wait_ge(va, 4)
                scalar.activation(fw1[:, :], fw0[:, :], ACT.Exp)        # texp from hmLc
                scalar.activation(fw2[:, :], fw6[:, :], ACT.Ln)         # ln p
                scalar.activation(fw3[:, :], fw6[:, :], ACT.Ln, bias=1.0, scale=-1.0)   # ln(1-p)
                scalar.activation(fw4[:, :], fw6[:, :], ACT.Square)     # p^2
                scalar.activation(fw5[:, :], fw6[:, :], ACT.Square, bias=1.0, scale=-1.0)  # (1-p)^2
                scalar.sem_inc(av, 1)

            @block.tensor
            def _(tensor):
                tensor.wait_ge(wrdy, 1)
                tensor.wait_ge(dma_in, 112)
                for g in range(NGRP):
                    b = g % NBANK
                    if g >= NBANK:
                        tensor.wait_ge(dv_s, g - NBANK + 1)
                    for c in range(4):
                        pixel0 = g * 512 + c * 128
                        tensor.matmul(
                            ps[b][:, c * 128 : (c + 1) * 128],
                            sQ[:, pixel0 : pixel0 + 128],
                            tmp[0:4, :],
                            start=True,
                            stop=True,
                        ).then_inc(pe_s, 1)
                # partial-sum reduction over partitions
                tensor.wait_ge(pt_s, 1)
                tensor.matmul(psp[:, :], ones, partials[:, :], start=True, stop=True).then_inc(pp_s, 1)
                tensor.wait_ge(d5, 16)
                tensor.matmul(psp[:, :], ones[0:8], agv[:, :], start=True, stop=True, skip_group_check=True).then_inc(pp_s, 1)

            @block.gpsimd
            def _(gpsimd):
                gpsimd.wait_ge(cell_s, 1)
                gpsimd.indirect_dma_start(
                    out=gb[:, :], out_offset=None,
                    in_=pb_d[:, :],
                    in_offset=bass.IndirectOffsetOnAxis(ap=cellg, axis=0),
                ).then_inc(g_s, 16)
                gpsimd.indirect_dma_start(
                    out=gc[:, :], out_offset=None,
                    in_=pc_d[:, :],
                    in_offset=bass.IndirectOffsetOnAxis(ap=cellg, axis=0),
                ).then_inc(g_s, 16)
                gpsimd.wait_ge(d4, 16)
                gpsimd.collective_compute(
                    "AllGather", ALU.bypass,
                    ins=[cc_in[:, :]], outs=[cc_out[:, :]],
                    replica_groups=[core_ids],
                ).then_inc(cc_s, 1)
                gpsimd.wait_ge(cc_s, 1)
                gpsimd.dma_start(out=agv[:, :], in_=cc_out[:, :]).then_inc(d5, 16)

            @block.vector
            def _(vraw):
                v = _DrainAfter(vraw)
                ts, stt = v.tensor_scalar, v.scalar_tensor_tensor
                x1, y1, x2, y2 = (sbb[:, i : i + 1] for i in range(4))
                v.memset(W5[:, :], 0.0)
                v.wait_ge(dma_in, 112)
                # ---- stage A: per-object quantities ----
                v.tensor_add(col["sumx"], x1, x2)
                ts(col["csx"], col["sumx"], 0.125, 0.5, op0=ALU.mult, op1=ALU.subtract)
                v.tensor_add(col["sumy"], y1, y2)
                ts(col["csy"], col["sumy"], 0.125, 0.5, op0=ALU.mult, op1=ALU.subtract)
                v.tensor_copy(gxi, col["csx"])   # round -> trunc(cx/4)
                v.tensor_copy(col["gxf"], gxi)
                ts(col["gxf"], col["gxf"], 0.0, 159.0, op0=ALU.max, op1=ALU.min)
                v.tensor_copy(gyi, col["csy"])
                v.tensor_copy(col["gyf"], gyi)
                ts(col["gyf"], col["gyf"], 0.0, 159.0, op0=ALU.max, op1=ALU.min)
                v.tensor_sub(col["bw"], x2, x1)
                v.tensor_sub(col["bh"], y2, y1)
                v.tensor_mul(col["area"], col["bw"], col["bh"])
                v.sem_inc(va, 1)                 # scalar: rq, dw, dh
                v.wait_ge(av, 1)
                ts(col["rm"], col["rq"], 2.0, 0.5, op0=ALU.max, op1=ALU.subtract)
                v.tensor_copy(sci[:, 3:4], col["rm"])
                v.tensor_copy(col["rf"], sci[:, 3:4])
                v.tensor_mul(col["r2"], col["rf"], col["rf"])
                v.reciprocal(col["rr"], col["r2"])
                ts(W5[:, 0:1], col["rr"], -2.0, None, op0=ALU.mult)        # nsc
                v.tensor_copy(col["nsc"], W5[:, 0:1])
                # validity
                v.tensor_copy(col["labf"], slab[:, :])
                ts(col["vlab"], col["labf"], 0.0, None, op0=ALU.is_ge)
                ts(col["vbw"], col["bw"], 0.0, None, op0=ALU.is_gt)
                ts(col["vbh"], col["bh"], 0.0, None, op0=ALU.is_gt)
                v.tensor_reduce(out=col["bbs"], in_=sbb[:, :], op=ALU.add, axis=mybir.AxisListType.X)
                ts(col["vbs"], col["bbs"], 0.0, None, op0=ALU.is_gt)
                v.tensor_mul(col["valid"], col["vlab"], col["vbw"])
                v.tensor_mul(col["valid"], col["valid"], col["vbh"])
                v.tensor_mul(col["valid"], col["valid"], col["vbs"])
                # W columns
                v.tensor_mul(col["gx2"], col["gxf"], col["gxf"])
                v.tensor_mul(col["gy2"], col["gyf"], col["gyf"])
                v.tensor_add(col["g2s"], col["gx2"], col["gy2"])
                v.tensor_mul(col["w3a"], col["nsc"], col["g2s"])
                ts(col["pen"], col["valid"], 1.0, 1e30, op0=ALU.subtract, op1=ALU.mult)
                v.tensor_add(W5[:, 3:4], col["w3a"], col["pen"])
                v.tensor_mul(col["gx2"], col["nsc"], col["gxf"])
                ts(W5[:, 1:2], col["gx2"], -2.0, None, op0=ALU.mult)
                v.tensor_mul(col["gy2"], col["nsc"], col["gyf"])
                ts(W5[:, 2:3], col["gy2"], -2.0, None, op0=ALU.mult)
                # cell index (float, exact) then int for gather
                stt(col["cellf"], col["gyf"], 160.0, col["gxf"], op0=ALU.mult, op1=ALU.add)
                v.tensor_add(col["cellgf"], col["cellf"], basef)
                v.tensor_copy(cellg, col["cellgf"])
                v.sem_inc(cell_s, 1)             # gpsimd can gather now
                # key = cell*43 + clipped label
                ts(col["labcf"], col["labf"], 0.0, 42.0, op0=ALU.max, op1=ALU.min)
                stt(col["keyf"], col["cellgf"], 43.0, col["labcf"], op0=ALU.mult, op1=ALU.add)
                # sentinelize invalid rows: cellS = cell*valid - (1-valid)*(1+i)
                v.tensor_mul(col["cva"], col["cellgf"], col["valid"])
                stt(col["sent"], col["valid"], 1.0, idxp1, op0=ALU.subtract, op1=ALU.mult)
                v.tensor_add(W5[:, 4:5], col["cva"], col["sent"])
                v.tensor_mul(col["cva"], col["keyf"], col["valid"])
                v.tensor_add(W5[:, 5:6], col["cva"], col["sent"])
                # transpose W5 -> tmp (rows 0-3 = Wt, 4 = cell row, 5 = key row)
                for c4 in range(4):
                    v.transpose(tmp[0:32, c4 * 32 : (c4 + 1) * 32], W5[c4 * 32 : (c4 + 1) * 32, 0:32])
                v.sem_inc(wrdy, 1)
                v.sem_inc(tr_s, 1)
                # box targets dx, dy
                v.tensor_sub(tbox[:, 0:1], col["csx"], col["gxf"])
                v.tensor_sub(tbox[:, 1:2], col["csy"], col["gyf"])
                # ---- dedup ----
                v.wait_ge(dma2, 64)
                ts(eqt[:, :], cbc[:, :], W5[:, 4:5], None, op0=ALU.is_equal)
                v.tensor_mul(junkm[:, :], eqt[:, :], sU[:, :])
                v.tensor_reduce(out=col["later1"], in_=junkm[:, :], op=ALU.max, axis=mybir.AxisListType.X)
                ts(col["cva"], col["later1"], -1.0, 1.0, op0=ALU.mult, op1=ALU.add)
                v.tensor_mul(partials[:, 2:3], col["valid"], col["cva"])   # kept
                v.tensor_copy(col["kept"], partials[:, 2:3])
                ts(eqt[:, :], kbc[:, :], W5[:, 5:6], None, op0=ALU.is_equal)
                v.tensor_mul(junkm[:, :], eqt[:, :], sU[:, :])
                v.tensor_reduce(out=col["later2"], in_=junkm[:, :], op=ALU.max, axis=mybir.AxisListType.X)
                ts(col["cva"], col["later2"], -1.0, 1.0, op0=ALU.mult, op1=ALU.add)
                v.tensor_mul(partials[:, 5:6], col["valid"], col["cva"])   # keep2
                v.tensor_copy(col["keep2"], partials[:, 5:6])
                # ---- box l1 (gathers needed) ----
                v.wait_ge(g_s, 32)
                v.tensor_sub(junk4[:, :], gb[:, :], tbox[:, :])
                ts(gb[:, :], junk4[:, :], -1.0, None, op0=ALU.mult)
                v.tensor_tensor(junk4[:, :], junk4[:, :], gb[:, :], op=ALU.max)
                v.tensor_reduce(out=col["l1r"], in_=junk4[:, :], op=ALU.add, axis=mybir.AxisListType.X)
                v.tensor_mul(partials[:, 3:4], col["l1r"], col["kept"])
                # ---- cls neg terms ----
                v.wait_ge(av, 2)                   # gcp = exp(-x)
                ts(gcp[:, :], gcp[:, :], 1.0, None, op0=ALU.add)
                v.reciprocal(gcp[:, :], gcp[:, :])
                ts(gcp[:, :], gcp[:, :], EPS, 1.0 - EPS, op0=ALU.max, op1=ALU.min)
                v.sem_inc(va, 1)                  # scalar: ln(1-p), p^2
                v.wait_ge(av, 3)
                stt(junk43[:, :], gc[:, :], -0.75, junk43[:, :], op0=ALU.mult, op1=ALU.mult, accum_out=col["negrow"])
                v.tensor_mul(partials[:, 4:5], col["negrow"], col["kept"])
                # ---- cls pos corrections ----
                ts(eqt[:, 0:NCLS], chm[:, :], col["labcf"], None, op0=ALU.is_equal)
                v.tensor_mul(junk43[:, :], gcp[:, :], eqt[:, 0:NCLS])
                v.tensor_reduce(out=col["plab"], in_=junk43[:, :], op=ALU.add, axis=mybir.AxisListType.X)
                v.sem_inc(va, 1)                  # scalar: ln/sq on plab
                v.wait_ge(av, 4)
                # pos_t = -0.25*(1-p)^2*ln p ; neg_t = -0.75*p^2*ln(1-p)
                stt(col["cva"], col["gy2"], -0.25, col["sumx"], op0=ALU.mult, op1=ALU.mult)
                stt(col["sent"], col["gx2"], -0.75, col["sumy"], op0=ALU.mult, op1=ALU.mult)
                v.tensor_sub(col["cva"], col["cva"], col["sent"])
                v.tensor_mul(partials[:, 6:7], col["cva"], col["keep2"])
                v.memset(partials[:, 7:8], 0.0)
                # ---- heatmap max-reduce pipeline ----
                last_inc = 0
                for g in range(NGRP):
                    b = g % NBANK
                    v.wait_ge(pe_s, 4 * (g + 1))
                    vraw.tensor_reduce(
                        out=hmL[:, :].rearrange("p (i f) -> p f i", i=2)[:, 4 * g : 4 * g + 4, :],
                        in_=ps[b][:, :].rearrange("p (a b m) -> p a b m", a=4, b=2),
                        op=ALU.max,
                        axis=mybir.AxisListType.X,
                    )
                    vraw.drain().then_inc(dv_s, 1)
                # ---- heat focal ----
                ts(fw0[:, :], hmL[:, :], -80.0, None, op0=ALU.max)          # hmLc
                ts(fw6[:, :], hmP[:, :], EPS, 1.0 - EPS, op0=ALU.max, op1=ALU.min)  # p
                v.sem_inc(va, 1)                  # scalar: texp, ln p, ln(1-p), p^2, (1-p)^2
                ts(hmP[:, :], fw0[:, :], THR, None, op0=ALU.is_ge)          # keep mask (reuse hmP)
                v.wait_ge(av, 5)
                v.tensor_mul(fw1[:, :], fw1[:, :], hmP[:, :])              # t
                stt(fw2[:, :], fw5[:, :], -0.25, fw2[:, :], op0=ALU.mult, op1=ALU.mult)  # A
                stt(fw3[:, :], fw4[:, :], 0.75, fw3[:, :], op0=ALU.mult, op1=ALU.mult)   # B'
                v.tensor_mul(fw4[:, :], fw2[:, :], fw1[:, :])              # X = A*t
                stt(fw5[:, :], fw1[:, :], 1.0, fw3[:, :], op0=ALU.subtract, op1=ALU.mult)  # Y = (t-1)*B'
                ts(fw0[:, :], fw1[:, :], 0.5, None, op0=ALU.is_gt)  # pos
                v.tensor_reduce(out=partials[:, 0:1], in_=fw0[:, :], op=ALU.add, axis=mybir.AxisListType.X)
                v.tensor_sub(fw2[:, :], fw4[:, :], fw5[:, :])          # X - Y
                v.tensor_mul(fw2[:, :], fw2[:, :], fw0[:, :])          # (X-Y)*pos
                v.tensor_add(fw6[:, :], fw2[:, :], fw5[:, :])
                v.tensor_reduce(out=partials[:, 1:2], in_=fw6[:, :], op=ALU.add, axis=mybir.AxisListType.X)
                v.sem_inc(pt_s, 1)
                # ---- partial vec out, collective, final ----
                v.wait_ge(pp_s, 1)
                v.tensor_copy(pvec[:, :], psp[:, :])
                v.sem_inc(pv_s, 1)
                v.wait_ge(pp_s, 2)
                v.tensor_copy(pvec[:, :], psp[:, :])
                gcol = [pvec[0:1, i : i + 1] for i in range(8)]
                r0, r1, r2 = sc[0:1, 0:1], sc[0:1, 1:2], sc[0:1, 2:3]
                r3, r4, r5 = sc[0:1, 3:4], sc[0:1, 4:5], sc[0:1, 5:6]
                ts(r0, gcol[0], 1.0, None, op0=ALU.max)
                v.reciprocal(r0, r0)
                v.tensor_mul(r0, gcol[1], r0)                    # heat_loss
                ts(r1, gcol[2], 1.0, None, op0=ALU.max)
                v.reciprocal(r1, r1)
                ts(r2, gcol[2], 1.0, None, op0=ALU.is_gt)         # ind
                v.tensor_mul(r1, gcol[3], r1)
                v.tensor_mul(r1, r1, r2)                         # box_loss
                ts(r3, gcol[5], 1.0, None, op0=ALU.max)
                v.reciprocal(r3, r3)
                v.tensor_add(r4, gcol[4], gcol[6])
                v.tensor_mul(r3, r4, r3)
                v.tensor_mul(r3, r3, r2)                         # cls_loss
                v.tensor_add(r5, r0, r1)
                v.tensor_add(res[:, :], r5, r3)
                v.sem_inc(fin_s, 1)

    return nc


_CACHE = {}


def _consts():
    j = np.arange(PIX)
    pix = (j % 128) * 200 + 4 * (j // 512) + (j % 512) // 128
    x = (pix % W).astype(np.float32)
    y = (pix // W).astype(np.float32)
    qgrid = np.stack([x * x + y * y, x, y, np.ones_like(x)]).astype(np.float32)
    utri = np.triu(np.ones((128, 128), dtype=np.float32), k=1)
    cvec = np.zeros((128, 8), dtype=np.float32)
    cvec[:, 0] = np.arange(128) + 1.0
    cvec[:, 1] = 1.0
    cvec[64:, 2] = PIX
    cvec[:, 4] = 1e-6
    cvec[:, 5] = -LN4
    chm = np.broadcast_to(np.arange(NCLS, dtype=np.float32), (128, NCLS)).copy()
    return qgrid, utri, cvec, chm


def kernel(pred_heatmap, pred_boxes, pred_classes, bboxes, labels):
    if "nc" not in _CACHE:
        _CACHE["nc"] = _build()
    nc = _CACHE["nc"]

    qgrid, utri, cvec, chm = _consts()
    pbt = np.ascontiguousarray(pred_boxes.transpose(0, 2, 3, 1).reshape(B, PIX, 4))
    pct = np.ascontiguousarray(pred_classes.transpose(0, 2, 3, 1).reshape(B, PIX, NCLS))
    hmf = np.ascontiguousarray(pred_heatmap.reshape(B, PIX)).astype(np.float32)
    lab32 = labels.astype(np.int32)

    in_maps = []
    for c in range(NC):
        s = slice(c * BPC, (c + 1) * BPC)
        in_maps.append({
            "hm": hmf[s],
            "pbt": pbt[s].reshape(BPC * PIX, 4),
            "pct": pct[s].reshape(BPC * PIX, NCLS),
            "bb": np.ascontiguousarray(bboxes[s]).astype(np.float32),
            "lab": np.ascontiguousarray(lab32[s]),
            "qgrid": qgrid, "utri": utri, "cvec": cvec, "chm": chm,
        })

    r = run_bass_kernel_spmd(nc, in_maps, list(range(NC)))
    return np.float32(np.asarray(r.results[0]["out"]).reshape(-1)[0])


if __name__ == "__main__":
    import reference
    inputs = reference.setup_inputs()
    inputs = {k: np.asarray(v) for k, v in inputs.items()}
    out = kernel(**inputs)
    exp = np.asarray(reference.reference(**{k: v for k, v in inputs.items()}))
    rel = abs(out - exp) / max(abs(exp), 1e-9)
    print("expected:", exp, "actual:", out, "rel:", rel)



# revision 4
# speedup vs baseline: 2.5252x; 2.5252x over previous
"""AnchorFreeLoss on 8 TRN2 NeuronCores (v2).

Strategy (data-parallel over batch, 2 images/core):
- Heatmap target: logG[pix, m] = -dist^2/(2*sigma_m^2) is affine in
  q(pix) = [x^2+y^2, x, y, 1]. Computed as ONE bf16 matmul per 512-pixel
  group: K=128 packs 4 pixel-chunks (block-diagonal weights, 32-row
  quadrant-aligned blocks) x 15 live rows (5 exact-bf16 q rows x 3-way
  bf16 split of the fp32 coefficients), so the PE streams 512 cols/group
  at full bf16 rate with near-fp32 accuracy (residual ~|W|*2^-24).
- PSUM -> bf16 SBUF copy on the Scalar engine, max-over-m reduce on DVE
  at 2x bf16 rate; 7-bank PSUM x 6-buffer bf16 ring pipeline.
- Box/class losses only touch object-center cells: gathered sparsely by
  GPSIMD indirect DMA from host-transposed [B*H*W, C] tables.
- Cell dedup via DRAM-bounce row broadcast + is_equal against an
  upper-triangular mask; last valid object wins (XLA scatter semantics).
- No collective: each core DMAs out its 8 partial sums; the final
  ~20-flop combine happens on host after gathering all cores.
"""

import sys
from contextlib import ExitStack

import numpy as np

if "/opt/trn_rl_repo" not in sys.path:
    sys.path.insert(0, "/opt/trn_rl_repo")

import ml_dtypes
from concourse import bass, mybir
from concourse.bass_utils import run_bass_kernel_spmd

F32 = mybir.dt.float32
BF16 = mybir.dt.bfloat16
I32 = mybir.dt.int32
ALU = mybir.AluOpType
ACT = mybir.ActivationFunctionType
AXX = mybir.AxisListType.X

B, M, H, W = 16, 64, 160, 160
NC = 8
BPC = B // NC          # 2 images per core
PIX = H * W            # 25600
NCLS = 43
EPS = 1e-7
THR = -8.0             # log-domain cutoff (= dist <= 2r since sigma = r/2)
NGRP = PIX // 512      # 50 groups of 512 pixels
NBANK = 7              # psum pipeline depth
NBFB = 6               # bf16 copy ring depth


def _build(debug=False):
    nc = bass.Bass()

    qg_d = nc.declare_dram_parameter("qg", [128, NGRP * 128], BF16, isOutput=False)
    hm_d = nc.declare_dram_parameter("hm2", [128, 400], F32, isOutput=False)
    pb_d = nc.declare_dram_parameter("pbt", [BPC * PIX, 4], F32, isOutput=False)
    pc_d = nc.declare_dram_parameter("pct", [BPC * PIX, NCLS], F32, isOutput=False)
    bb_d = nc.declare_dram_parameter("bb", [BPC, M, 4], F32, isOutput=False)
    lab_d = nc.declare_dram_parameter("lab", [BPC, M], I32, isOutput=False)
    ut_d = nc.declare_dram_parameter("utri", [128, 128], F32, isOutput=False)
    cv_d = nc.declare_dram_parameter("cvec", [128, 8], F32, isOutput=False)
    ch_d = nc.declare_dram_parameter("chm", [128, NCLS], F32, isOutput=False)
    out_d = nc.declare_dram_parameter("out", [1, 8], F32, isOutput=True)
    dbg = {}
    if debug:
        for nm, shp, dt in [("d_partials", [128, 8], F32), ("d_sc", [128, 48], F32),
                            ("d_hmL", [128, 400], BF16), ("d_w5", [128, 32], F32)]:
            dbg[nm] = nc.declare_dram_parameter(nm, shp, dt, isOutput=True)

    dbc = nc.dram_tensor("dbc", [2, 128], F32)

    es = ExitStack()
    dS = es.enter_context(nc.semaphore("dS"))        # small input dmas
    dH = es.enter_context(nc.semaphore("dH"))        # hm2 dma
    dQ = es.enter_context(nc.semaphore("dQ"))        # qgrid dma
    dB = es.enter_context(nc.semaphore("dB"))        # broadcast bounce dmas
    dO = es.enter_context(nc.semaphore("dO"))        # output dma
    va = es.enter_context(nc.semaphore("va"))        # vector -> scalar
    av = es.enter_context(nc.semaphore("av"))        # scalar -> vector
    cell_s = es.enter_context(nc.semaphore("cell_s"))
    g_s = es.enter_context(nc.semaphore("g_s"))
    tr_s = es.enter_context(nc.semaphore("tr_s"))
    wrdy = es.enter_context(nc.semaphore("wrdy"))
    pe_s = es.enter_context(nc.semaphore("pe_s"))
    as_s = es.enter_context(nc.semaphore("as_s"))
    dv_s = es.enter_context(nc.semaphore("dv_s"))
    pt_s = es.enter_context(nc.semaphore("pt_s"))
    pp_s = es.enter_context(nc.semaphore("pp_s"))
    pv_s = es.enter_context(nc.semaphore("pv_s"))

    sQ = es.enter_context(nc.sbuf_tensor("sQ", [128, NGRP * 128], BF16))
    wt128 = es.enter_context(nc.sbuf_tensor("wt128", [128, 512], BF16))
    wb = es.enter_context(nc.sbuf_tensor("wb", [128, 16], BF16))
    tmp = es.enter_context(nc.sbuf_tensor("tmp", [32, 128], F32))
    sU = es.enter_context(nc.sbuf_tensor("sU", [128, 128], F32))
    cvec = es.enter_context(nc.sbuf_tensor("cvec_s", [128, 8], F32))
    chm = es.enter_context(nc.sbuf_tensor("chm_s", [128, NCLS], F32))
    sbb = es.enter_context(nc.sbuf_tensor("sbb", [128, 4], F32))
    slab = es.enter_context(nc.sbuf_tensor("slab", [128, 1], I32))
    W5 = es.enter_context(nc.sbuf_tensor("W5", [128, 32], F32))
    sc = es.enter_context(nc.sbuf_tensor("sc", [128, 48], F32))
    sci = es.enter_context(nc.sbuf_tensor("sci", [128, 4], I32))
    hmP = es.enter_context(nc.sbuf_tensor("hmP", [128, 400], F32))
    lnp = es.enter_context(nc.sbuf_tensor("lnp", [128, 400], F32))
    ln1p = es.enter_context(nc.sbuf_tensor("ln1p", [128, 400], F32))
    p2 = es.enter_context(nc.sbuf_tensor("p2", [128, 400], F32))
    q2 = es.enter_context(nc.sbuf_tensor("q2", [128, 400], F32))
    texp = es.enter_context(nc.sbuf_tensor("texp", [128, 400], F32))
    fw0 = es.enter_context(nc.sbuf_tensor("fw0", [128, 400], F32))
    keepm = es.enter_context(nc.sbuf_tensor("keepm", [128, 400], F32))
    hmL = es.enter_context(nc.sbuf_tensor("hmL", [128, 400], BF16))
    bfb = [es.enter_context(nc.sbuf_tensor(f"bfb{i}", [128, 512], BF16))
           for i in range(NBFB)]
    cbc = es.enter_context(nc.sbuf_tensor("cbc", [128, 128], F32))
    lbc = es.enter_context(nc.sbuf_tensor("lbc", [128, 128], F32))
    eqt1 = es.enter_context(nc.sbuf_tensor("eqt1", [128, 128], F32))
    eq2 = es.enter_context(nc.sbuf_tensor("eq2", [128, 128], F32))
    jm = es.enter_context(nc.sbuf_tensor("jm", [128, 128], F32))
    tbox = es.enter_context(nc.sbuf_tensor("tbox", [128, 4], F32))
    gb = es.enter_context(nc.sbuf_tensor("gb", [128, 4], F32))
    junk4 = es.enter_context(nc.sbuf_tensor("junk4", [128, 4], F32))
    gc = es.enter_context(nc.sbuf_tensor("gc", [128, NCLS], F32))
    gcp = es.enter_context(nc.sbuf_tensor("gcp", [128, NCLS], F32))
    junk43 = es.enter_context(nc.sbuf_tensor("junk43", [128, NCLS], F32))
    partials = es.enter_context(nc.sbuf_tensor("partials", [128, 8], F32))
    pvec = es.enter_context(nc.sbuf_tensor("pvec", [1, 8], F32))

    ps = [es.enter_context(nc.psum_tensor(f"ps{i}", [128, 512], F32))
          for i in range(NBANK)]
    psp = es.enter_context(nc.psum_tensor("psp", [1, 8], F32))

    with es:
        names = [
            "sumx", "csx", "sumy", "csy", "gxf", "gyf", "bw", "bh", "area",
            "rq", "rm", "rf", "r2", "rr", "labf", "vlab", "vbw", "vbh",
            "bbs", "vbs", "valid", "gx2", "gy2", "g2s", "pen", "cellf",
            "cellgf", "labcf", "cva", "sent", "later1", "later2", "c1", "c2",
            "l1r", "negrow", "plab", "lnpl", "ln1pl", "p2pl", "q2pl", "t1",
        ]
        col = {n: sc[:, i: i + 1] for i, n in enumerate(names)}
        idxp1 = cvec[:, 0:1]
        ones = cvec[:, 1:2]
        basef = cvec[:, 2:3]
        nc.const_aps.aps[(F32, 0.0)] = cvec[:, 3:4]
        nc.const_aps.aps[(F32, 1.0)] = cvec[:, 1:2]
        nc.const_aps.aps[(F32, 1e-6)] = cvec[:, 4:5]
        gxi = sci[:, 0:1]
        gyi = sci[:, 1:2]
        ri = sci[:, 2:3]
        cellg = sci[:, 3:4]
        # W5 columns: 0-14 bf16-rounded splits (A,B,C x [w1,w1,w2,w3,w4]),
        # 15 cellS, 16-20 original W, 21-25 resA, 26-30 resB, 31 labS
        nsc = W5[:, 16:17]
        cellS = W5[:, 15:16]
        labS = W5[:, 31:32]

        with nc.Block() as block:

            @block.sync
            def _(sync):
                sync.dma_start(out=sbb[:, :], in_=bb_d[:, :, :].rearrange("a m c -> (a m) c")).then_inc(dS, 16)
                sync.dma_start(out=slab[:, :], in_=lab_d[:, :].rearrange("a m -> (a m)").unsqueeze(1)).then_inc(dS, 16)
                sync.dma_start(out=cvec[:, :], in_=cv_d[:, :]).then_inc(dS, 16)
                sync.dma_start(out=chm[:, :], in_=ch_d[:, :]).then_inc(dS, 16)
                sync.dma_start(out=sU[:, :], in_=ut_d[:, :]).then_inc(dS, 16)
                sync.dma_start(out=hmP[:, :], in_=hm_d[:, :]).then_inc(dH, 16)
                sync.dma_start(out=sQ[:, :], in_=qg_d[:, :]).then_inc(dQ, 16)
                # cellS/labS broadcast via DRAM bounce
                sync.wait_ge(tr_s, 1)
                sync.dma_start(out=dbc[0:1, :], in_=tmp[15:16, :]).then_inc(dB, 16)
                sync.dma_start(out=dbc[1:2, :], in_=tmp[31:32, :]).then_inc(dB, 16)
                sync.wait_ge(dB, 32)
                sync.dma_start(out=cbc[:, :], in_=dbc[0:1, :].to_broadcast([128, 128])).then_inc(dB, 16)
                sync.dma_start(out=lbc[:, :], in_=dbc[1:2, :].to_broadcast([128, 128])).then_inc(dB, 16)
                # partial-sum vector out
                sync.wait_ge(pv_s, 1)
                sync.dma_start(out=out_d[:, :], in_=pvec[:, :]).then_inc(dO, 16)
                ndO = 16
                if debug:
                    for nm, t in [("d_partials", partials), ("d_sc", sc),
                                  ("d_hmL", hmL), ("d_w5", W5)]:
                        sync.dma_start(out=dbg[nm][:, :], in_=t[:, :]).then_inc(dO, 16)
                        ndO += 16
                sync.wait_ge(dO, ndO)

            @block.gpsimd
            def _(gpsimd):
                gpsimd.wait_ge(cell_s, 1)
                gpsimd.indirect_dma_start(
                    out=gb[:, :], out_offset=None,
                    in_=pb_d[:, :],
                    in_offset=bass.IndirectOffsetOnAxis(ap=cellg, axis=0),
                ).then_inc(g_s, 16)
                gpsimd.indirect_dma_start(
                    out=gc[:, :], out_offset=None,
                    in_=pc_d[:, :],
                    in_offset=bass.IndirectOffsetOnAxis(ap=cellg, axis=0),
                ).then_inc(g_s, 16)

            @block.tensor
            def _(tensor):
                tensor.wait_ge(wrdy, 1)
                tensor.wait_ge(dQ, 16)
                for g in range(NGRP):
                    b = g % NBANK
                    if g >= NBANK:
                        tensor.wait_ge(as_s, g - NBANK + 1)
                    tensor.matmul(
                        ps[b][:, :],
                        sQ[:, g * 128: (g + 1) * 128],
                        wt128[:, :],
                        start=True, stop=True,
                    ).then_inc(pe_s, 1)
                # partial-sum reduction over partitions
                tensor.wait_ge(pt_s, 1)
                tensor.matmul(psp[:, :], ones, partials[:, :], start=True,
                              stop=True).then_inc(pp_s, 1)

            @block.scalar
            def _(scalar):
                # stage A transcendentals: r = sqrt(area)/4, dw/dh
                scalar.wait_ge(va, 1)
                scalar.activation(col["rq"], col["area"], ACT.Sqrt, scale=0.0625)
                scalar.activation(tbox[:, 2:3], col["bw"], ACT.Ln, bias=1e-6, scale=0.25)
                scalar.activation(tbox[:, 3:4], col["bh"], ACT.Ln, bias=1e-6, scale=0.25)
                scalar.drain().then_inc(av, 1)                      # av1
                # pred-heatmap transcendentals (all read clipped hmP only)
                scalar.wait_ge(va, 2)
                scalar.activation(lnp[:, :], hmP[:, :], ACT.Ln)
                scalar.activation(ln1p[:, :], hmP[:, :], ACT.Ln, bias=1.0, scale=-1.0)
                scalar.activation(p2[:, :], hmP[:, :], ACT.Square)
                scalar.activation(q2[:, :], hmP[:, :], ACT.Square, bias=1.0, scale=-1.0)
                scalar.drain().then_inc(av, 1)                      # av2
                # cls sigmoid numerator (gathers done well before matmuls ramp)
                scalar.wait_ge(g_s, 32)
                scalar.activation(gcp[:, :], gc[:, :], ACT.Exp, scale=-1.0)
                scalar.drain().then_inc(av, 1)                      # av3
                # psum -> bf16 ring, with mid-stream one-shot work
                for g in range(NGRP):
                    if g == 20:
                        scalar.wait_ge(va, 3)
                        scalar.activation(junk43[:, :], gcp[:, :], ACT.Ln, bias=1.0, scale=-1.0)
                        scalar.activation(gc[:, :], gcp[:, :], ACT.Square)
                        scalar.drain().then_inc(av, 1)              # av4
                    if g == 30:
                        scalar.wait_ge(va, 4)
                        scalar.activation(col["lnpl"], col["plab"], ACT.Ln)
                        scalar.activation(col["ln1pl"], col["plab"], ACT.Ln, bias=1.0, scale=-1.0)
                        scalar.activation(col["p2pl"], col["plab"], ACT.Square)
                        scalar.activation(col["q2pl"], col["plab"], ACT.Square, bias=1.0, scale=-1.0)
                        scalar.drain().then_inc(av, 1)              # av5
                    scalar.wait_ge(pe_s, g + 1)
                    if g >= NBFB:
                        scalar.wait_ge(dv_s, g - NBFB + 1)
                    scalar.activation(bfb[g % NBFB][:, :], ps[g % NBANK][:, :],
                                      ACT.Copy).then_inc(as_s, 1)
                # heat target exp
                scalar.wait_ge(va, 5)
                scalar.activation(texp[:, :], fw0[:, :], ACT.Exp).then_inc(av, 1)  # av6

            @block.vector
            def _(v):
                ts, stt = v.tensor_scalar, v.scalar_tensor_tensor
                x1, y1, x2, y2 = (sbb[:, i: i + 1] for i in range(4))

                def slot(*thunks):
                    for t in thunks:
                        t()
                    v.drain()

                v.memset(wt128[:, :], 0.0)
                v.memset(partials[:, :], 0.0)
                v.wait_ge(dS, 80)
                # ---- stage A ----
                slot(lambda: v.tensor_add(col["sumx"], x1, x2),
                     lambda: v.tensor_add(col["sumy"], y1, y2))
                slot(lambda: ts(col["csx"], col["sumx"], 0.125, 0.5, op0=ALU.mult, op1=ALU.subtract),
                     lambda: ts(col["csy"], col["sumy"], 0.125, 0.5, op0=ALU.mult, op1=ALU.subtract))
                slot(lambda: v.tensor_copy(gxi, col["csx"]),
                     lambda: v.tensor_copy(gyi, col["csy"]))
                slot(lambda: v.tensor_copy(col["gxf"], gxi),
                     lambda: v.tensor_copy(col["gyf"], gyi))
                slot(lambda: ts(col["gxf"], col["gxf"], 0.0, 159.0, op0=ALU.max, op1=ALU.min),
                     lambda: ts(col["gyf"], col["gyf"], 0.0, 159.0, op0=ALU.max, op1=ALU.min))
                slot(lambda: v.tensor_sub(col["bw"], x2, x1),
                     lambda: v.tensor_sub(col["bh"], y2, y1))
                slot(lambda: v.tensor_mul(col["area"], col["bw"], col["bh"]),
                     lambda: v.tensor_copy(col["labf"], slab[:, :]))
                v.sem_inc(va, 1)                                    # va1
                slot(lambda: v.tensor_reduce(out=col["bbs"], in_=sbb[:, :], op=ALU.add, axis=AXX),
                     lambda: ts(col["vlab"], col["labf"], 0.0, None, op0=ALU.is_ge))
                slot(lambda: ts(col["vbw"], col["bw"], 0.0, None, op0=ALU.is_gt),
                     lambda: ts(col["vbh"], col["bh"], 0.0, None, op0=ALU.is_gt))
                slot(lambda: ts(col["vbs"], col["bbs"], 0.0, None, op0=ALU.is_gt),
                     lambda: v.tensor_mul(col["gx2"], col["gxf"], col["gxf"]))
                slot(lambda: v.tensor_mul(col["valid"], col["vlab"], col["vbw"]),
                     lambda: v.tensor_mul(col["gy2"], col["gyf"], col["gyf"]))
                slot(lambda: v.tensor_mul(col["valid"], col["valid"], col["vbh"]),
                     lambda: v.tensor_add(col["g2s"], col["gx2"], col["gy2"]))
                slot(lambda: v.tensor_mul(col["valid"], col["valid"], col["vbs"]),
                     lambda: stt(col["cellf"], col["gyf"], 160.0, col["gxf"], op0=ALU.mult, op1=ALU.add))
                slot(lambda: ts(col["pen"], col["valid"], 1.0, 1e30, op0=ALU.subtract, op1=ALU.mult),
                     lambda: v.tensor_add(col["cellgf"], col["cellf"], basef))
                slot(lambda: v.tensor_copy(cellg, col["cellgf"]),
                     lambda: v.tensor_sub(tbox[:, 0:1], col["csx"], col["gxf"]))
                v.sem_inc(cell_s, 1)
                slot(lambda: v.tensor_sub(tbox[:, 1:2], col["csy"], col["gyf"]),
                     lambda: ts(col["labcf"], col["labf"], 0.0, 42.0, op0=ALU.max, op1=ALU.min))
                v.wait_ge(av, 1)
                slot(lambda: ts(col["rm"], col["rq"], 2.0, 0.5, op0=ALU.max, op1=ALU.subtract),
                     lambda: v.tensor_mul(col["cva"], col["cellgf"], col["valid"]))
                slot(lambda: v.tensor_copy(ri, col["rm"]),
                     lambda: stt(col["sent"], col["valid"], 1.0, idxp1, op0=ALU.subtract, op1=ALU.mult))
                slot(lambda: v.tensor_copy(col["rf"], ri),
                     lambda: v.tensor_add(cellS, col["cva"], col["sent"]))
                slot(lambda: v.tensor_mul(col["r2"], col["rf"], col["rf"]),
                     lambda: v.tensor_copy(labS, col["labcf"]))
                slot(lambda: v.reciprocal(col["rr"], col["r2"]))
                slot(lambda: ts(nsc, col["rr"], -2.0, None, op0=ALU.mult))
                slot(lambda: v.tensor_copy(W5[:, 17:18], nsc),
                     lambda: stt(W5[:, 18:19], nsc, -2.0, col["gxf"], op0=ALU.mult, op1=ALU.mult),
                     lambda: stt(W5[:, 19:20], nsc, -2.0, col["gyf"], op0=ALU.mult, op1=ALU.mult),
                     lambda: v.tensor_mul(col["t1"], nsc, col["g2s"]))
                slot(lambda: v.tensor_add(W5[:, 20:21], col["t1"], col["pen"]))
                # 3-way bf16 split: cols 16-20 -> bf16-backed cols 0-4,5-9,10-14
                slot(lambda: v.tensor_copy(wb[:, 0:5], W5[:, 16:21]))
                slot(lambda: v.tensor_copy(W5[:, 0:5], wb[:, 0:5]))
                slot(lambda: v.tensor_sub(W5[:, 21:26], W5[:, 16:21], W5[:, 0:5]))
                slot(lambda: v.tensor_copy(wb[:, 5:10], W5[:, 21:26]))
                slot(lambda: v.tensor_copy(W5[:, 5:10], wb[:, 5:10]))
                slot(lambda: v.tensor_sub(W5[:, 26:31], W5[:, 21:26], W5[:, 5:10]))
                slot(lambda: v.tensor_copy(wb[:, 10:15], W5[:, 26:31]))
                slot(lambda: v.tensor_copy(W5[:, 10:15], wb[:, 10:15]))
                # transpose W5 -> tmp (rows 0-14 = W' splits, 15 cellS, 31 labS)
                slot(*[
                    (lambda c4=c4: v.transpose(
                        tmp[0:32, c4 * 32: (c4 + 1) * 32],
                        W5[c4 * 32: (c4 + 1) * 32, 0:32]))
                    for c4 in range(4)
                ])
                v.sem_inc(tr_s, 1)
                # block-diagonal wt128 (quadrant-aligned cross-partition casts)
                slot(*[
                    (lambda c=c: v.tensor_copy(
                        wt128[32 * c: 32 * c + 15, 128 * c: 128 * (c + 1)],
                        tmp[0:15, :]))
                    for c in range(4)
                ])
                v.sem_inc(wrdy, 1)
                # ---- stage B (overlapping matmul phase) ----
                v.wait_ge(dH, 16)
                slot(lambda: ts(hmP[:, :], hmP[:, :], EPS, 1.0 - EPS, op0=ALU.max, op1=ALU.min))
                v.sem_inc(va, 1)                                    # va2
                # box l1 prefix (gathers done; kept-mult comes after dedup)
                v.wait_ge(g_s, 32)
                slot(lambda: v.tensor_sub(junk4[:, :], gb[:, :], tbox[:, :]))
                slot(lambda: ts(gb[:, :], junk4[:, :], -1.0, None, op0=ALU.mult))
                slot(lambda: v.tensor_tensor(junk4[:, :], junk4[:, :], gb[:, :], op=ALU.max))
                slot(lambda: v.tensor_reduce(out=col["l1r"], in_=junk4[:, :], op=ALU.add, axis=AXX))
                # heatmap reduce pipeline + interleaved one-shot work
                for g in range(NGRP):
                    v.wait_ge(as_s, g + 1)
                    v.tensor_reduce(
                        out=hmL[:, :].rearrange("p (i f) -> p f i", i=2)[:, 4 * g: 4 * g + 4, :],
                        in_=bfb[g % NBFB][:, :].rearrange("p (a b m) -> p a b m", a=4, b=2),
                        op=ALU.max,
                        axis=AXX,
                    ).then_inc(dv_s, 1)
                    if g == 6:
                        # dedup: kept (per cell) and keep2 (per cell+label)
                        v.wait_ge(dB, 64)
                        slot(lambda: ts(eqt1[:, :], cbc[:, :], cellS, None, op0=ALU.is_equal),
                             lambda: ts(eq2[:, :], lbc[:, :], col["labcf"], None, op0=ALU.is_equal))
                        slot(lambda: v.tensor_mul(eq2[:, :], eq2[:, :], eqt1[:, :]),
                             lambda: v.tensor_mul(jm[:, :], eqt1[:, :], sU[:, :]))
                        slot(lambda: v.tensor_mul(eqt1[:, :], eq2[:, :], sU[:, :]),
                             lambda: v.tensor_reduce(out=col["later1"], in_=jm[:, :], op=ALU.max, axis=AXX))
                        slot(lambda: v.tensor_reduce(out=col["later2"], in_=eqt1[:, :], op=ALU.max, axis=AXX),
                             lambda: ts(col["c1"], col["later1"], -1.0, 1.0, op0=ALU.mult, op1=ALU.add))
                        slot(lambda: ts(col["c2"], col["later2"], -1.0, 1.0, op0=ALU.mult, op1=ALU.add),
                             lambda: v.tensor_mul(partials[:, 2:3], col["valid"], col["c1"]))
                        slot(lambda: v.tensor_mul(partials[:, 5:6], col["valid"], col["c2"]),
                             lambda: v.tensor_mul(partials[:, 3:4], col["l1r"], partials[:, 2:3]))
                    if g == 10:
                        # cls sigmoid chain (scalar exp done pre-loop: av3)
                        v.wait_ge(av, 3)
                        slot(lambda: ts(gcp[:, :], gcp[:, :], 1.0, None, op0=ALU.add))
                        slot(lambda: v.reciprocal(gcp[:, :], gcp[:, :]))
                        slot(lambda: ts(gcp[:, :], gcp[:, :], EPS, 1.0 - EPS, op0=ALU.max, op1=ALU.min))
                        v.sem_inc(va, 1)                            # va3
                    if g == 24:
                        # cls negative rows (needs scalar av4: ln(1-p), p^2)
                        v.wait_ge(av, 4)
                        slot(lambda: stt(junk43[:, :], gc[:, :], -0.75, junk43[:, :],
                                         op0=ALU.mult, op1=ALU.mult, accum_out=col["negrow"]),
                             lambda: ts(eqt1[:, 0:NCLS], chm[:, :], col["labcf"], None, op0=ALU.is_equal))
                        slot(lambda: v.tensor_mul(partials[:, 4:5], col["negrow"], partials[:, 2:3]),
                             lambda: v.tensor_mul(junk43[:, :], gcp[:, :], eqt1[:, 0:NCLS]))
                        slot(lambda: v.tensor_reduce(out=col["plab"], in_=junk43[:, :], op=ALU.add, axis=AXX))
                        v.sem_inc(va, 1)                            # va4
                    if g == 36:
                        # cls positive corrections (needs scalar av5)
                        v.wait_ge(av, 5)
                        slot(lambda: stt(col["cva"], col["q2pl"], -0.25, col["lnpl"], op0=ALU.mult, op1=ALU.mult),
                             lambda: stt(col["sent"], col["p2pl"], -0.75, col["ln1pl"], op0=ALU.mult, op1=ALU.mult))
                        slot(lambda: v.tensor_sub(col["cva"], col["cva"], col["sent"]))
                        slot(lambda: v.tensor_mul(partials[:, 6:7], col["cva"], partials[:, 5:6]))
                # ---- heat focal tail ----
                slot(lambda: ts(fw0[:, :], hmL[:, :], -80.0, None, op0=ALU.max))
                v.sem_inc(va, 1)                                    # va5 (scalar: texp)
                slot(lambda: ts(keepm[:, :], fw0[:, :], THR, None, op0=ALU.is_ge))
                v.wait_ge(av, 6)
                slot(lambda: v.tensor_mul(texp[:, :], texp[:, :], keepm[:, :]))
                slot(lambda: stt(lnp[:, :], q2[:, :], -0.25, lnp[:, :], op0=ALU.mult, op1=ALU.mult),
                     lambda: stt(ln1p[:, :], p2[:, :], 0.75, ln1p[:, :], op0=ALU.mult, op1=ALU.mult))
                slot(lambda: v.tensor_mul(lnp[:, :], lnp[:, :], texp[:, :]),
                     lambda: stt(ln1p[:, :], texp[:, :], 1.0, ln1p[:, :], op0=ALU.subtract, op1=ALU.mult))
                slot(lambda: ts(q2[:, :], texp[:, :], 0.5, None, op0=ALU.is_gt))
                slot(lambda: v.tensor_reduce(out=partials[:, 0:1], in_=q2[:, :], op=ALU.add, axis=AXX),
                     lambda: v.tensor_sub(lnp[:, :], lnp[:, :], ln1p[:, :]))
                slot(lambda: v.tensor_mul(lnp[:, :], lnp[:, :], q2[:, :]))
                slot(lambda: v.tensor_add(lnp[:, :], lnp[:, :], ln1p[:, :]))
                slot(lambda: v.tensor_reduce(out=partials[:, 1:2], in_=lnp[:, :], op=ALU.add, axis=AXX))
                v.sem_inc(pt_s, 1)
                # ---- partial vec out ----
                v.wait_ge(pp_s, 1)
                slot(lambda: v.tensor_copy(pvec[:, :], psp[:, :]))
                v.sem_inc(pv_s, 1)

    return nc


_CACHE = {}


def _consts():
    j = np.arange(PIX)
    x = (j % W).astype(np.float32)
    y = (j // W).astype(np.float32)
    q1 = x * x + y * y
    q1hi = q1.astype(ml_dtypes.bfloat16).astype(np.float32)
    q1lo = q1 - q1hi
    onesv = np.ones_like(x)
    q5 = np.stack([q1hi, q1lo, x, y, onesv])           # [5, PIX] all bf16-exact
    # qg[32 c + 5 s + r, 128 g + p] = q5[r, 512 g + 128 c + p]
    q5r = q5.reshape(5, NGRP, 4, 128)                  # [r, g, c, p]
    qg = np.zeros((128, NGRP * 128), np.float32)
    for c in range(4):
        for s in range(3):
            qg[32 * c + 5 * s: 32 * c + 5 * s + 5, :] = (
                q5r[:, :, c, :].reshape(5, NGRP * 128))
    qg = qg.astype(ml_dtypes.bfloat16)
    utri = np.triu(np.ones((128, 128), dtype=np.float32), k=1)
    cvec = np.zeros((128, 8), dtype=np.float32)
    cvec[:, 0] = np.arange(128) + 1.0
    cvec[:, 1] = 1.0
    cvec[64:, 2] = PIX
    cvec[:, 4] = 1e-6
    chm = np.broadcast_to(np.arange(NCLS, dtype=np.float32), (128, NCLS)).copy()
    return qg, utri, cvec, chm


def _prep(pred_heatmap, pred_boxes, pred_classes, bboxes, labels):
    qg, utri, cvec, chm = _consts()
    pbt = np.ascontiguousarray(
        pred_boxes.transpose(0, 2, 3, 1).reshape(B, PIX, 4)).astype(np.float32)
    pct = np.ascontiguousarray(
        pred_classes.transpose(0, 2, 3, 1).reshape(B, PIX, NCLS)).astype(np.float32)
    # hm2[p, img*200 + f] = hm[img, 128 f + p]
    hmr = np.asarray(pred_heatmap, np.float32).reshape(B, 200, 128)
    lab32 = np.asarray(labels).astype(np.int32)
    in_maps = []
    for c in range(NC):
        s = slice(c * BPC, (c + 1) * BPC)
        hm2 = np.ascontiguousarray(hmr[s].transpose(2, 0, 1).reshape(128, 400))
        in_maps.append({
            "qg": qg,
            "hm2": hm2,
            "pbt": pbt[s].reshape(BPC * PIX, 4),
            "pct": pct[s].reshape(BPC * PIX, NCLS),
            "bb": np.ascontiguousarray(bboxes[s]).astype(np.float32),
            "lab": np.ascontiguousarray(lab32[s]),
            "utri": utri, "cvec": cvec, "chm": chm,
        })
    return in_maps


def _combine(pvecs):
    P = np.sum(np.stack(pvecs, 0), axis=0, dtype=np.float64).astype(np.float32)
    heat = P[1] / max(P[0], np.float32(1.0))
    num_pos = max(P[2], np.float32(1.0))
    box = P[3] / num_pos if P[2] > 1.0 else np.float32(0.0)
    cls = (P[4] + P[6]) / max(P[5], np.float32(1.0)) if P[2] > 1.0 else np.float32(0.0)
    return np.float32(heat + box + cls)


def kernel(pred_heatmap, pred_boxes, pred_classes, bboxes, labels):
    if "nc" not in _CACHE:
        _CACHE["nc"] = _build()
    nc = _CACHE["nc"]
    in_maps = _prep(pred_heatmap, pred_boxes, pred_classes, bboxes, labels)
    r = run_bass_kernel_spmd(nc, in_maps, list(range(NC)))
    pvecs = [np.asarray(r.results[c]["out"]).reshape(8) for c in range(NC)]
    return _combine(pvecs)


if __name__ == "__main__":
    import reference
    inputs = reference.setup_inputs()
    inputs = {k: np.asarray(v) for k, v in inputs.items()}
    out = kernel(**inputs)
    exp = np.asarray(reference.reference(**inputs))
    rel = abs(out - exp) / max(abs(exp), 1e-9)
    print("expected:", exp, "actual:", out, "rel:", rel)


# revision 8
# speedup vs baseline: 2.5745x; 1.0195x over previous
"""AnchorFreeLoss on 8 TRN2 NeuronCores (v3).

Strategy (data-parallel over batch, 2 images/core):
- Host prep (tiny [B,M] per-object math, like the qgrid/NHWC transposes):
  per-object centers/radii -> packed block-diagonal bf16 coefficient
  matrix wt128 (3-way bf16 split of fp32 coefficients, quadrant-aligned),
  gather offsets, box targets, class one-hots, and scatter dedup flags
  (last valid object per cell wins, matching XLA scatter semantics).
- Device heatmap target: logG[pix, m] = -dist^2/(2*sigma^2) is affine in
  q(pix) = [x^2+y^2, x, y, 1]: ONE bf16 matmul per 512-pixel group
  (K=128 packs 4 pixel-chunks x 15 live rows) into two 4-bank PSUM slabs;
  DVE max-reduces a whole 2048-col slab per instruction. Cutoff dist<=2r
  equals logG >= -8 exactly (sigma = r/2).
- Focal transcendentals on the Scalar engine (incl. per-slab exp of the
  log-heatmap); focal algebra + reductions on DVE.
- Box/class losses only touch object-center cells: GPSIMD indirect-DMA
  gathers from host-transposed [B*H*W, C] tables.
- No collective: GPSIMD cross-lane-reduces the per-partition partial
  sums; each core DMAs out 8 floats; host combines (the unshard step).
"""

import sys
from contextlib import ExitStack

import numpy as np

if "/opt/trn_rl_repo" not in sys.path:
    sys.path.insert(0, "/opt/trn_rl_repo")

import ml_dtypes
from concourse import bass, mybir
from concourse.bass_utils import run_bass_kernel_spmd

F32 = mybir.dt.float32
BF16 = mybir.dt.bfloat16
I32 = mybir.dt.int32
ALU = mybir.AluOpType
ACT = mybir.ActivationFunctionType
AXX = mybir.AxisListType.X

B, M, H, W = 16, 64, 160, 160
NC = 8
BPC = B // NC          # 2 images per core
PIX = H * W            # 25600
NCLS = 43
EPS = 1e-7
THR = -8.0             # log-domain cutoff (= dist <= 2r since sigma = r/2)
NGRP = PIX // 512      # 50 groups of 512 pixels
NSLAB = (NGRP + 3) // 4  # 13 slabs (last one half-size)


def _build(debug=False):
    nc = bass.Bass()

    qg_d = nc.declare_dram_parameter("qg", [128, NGRP * 128], BF16, isOutput=False)
    wt_d = nc.declare_dram_parameter("wt", [128, 512], BF16, isOutput=False)
    hm_d = nc.declare_dram_parameter("hm2", [128, 400], F32, isOutput=False)
    pb_d = nc.declare_dram_parameter("pbt", [BPC * PIX, 4], F32, isOutput=False)
    pc_d = nc.declare_dram_parameter("pct", [BPC * PIX, NCLS], F32, isOutput=False)
    cg_d = nc.declare_dram_parameter("cellg", [128, 1], I32, isOutput=False)
    tb_d = nc.declare_dram_parameter("tboxd", [128, 4], F32, isOutput=False)
    oh_d = nc.declare_dram_parameter("onehot", [128, NCLS], F32, isOutput=False)
    pi_d = nc.declare_dram_parameter("parti", [128, 8], F32, isOutput=False)
    cv_d = nc.declare_dram_parameter("cvec", [128, 8], F32, isOutput=False)
    out_d = nc.declare_dram_parameter("out", [1, 8], F32, isOutput=True)
    dbg = {}
    if debug:
        for nm, shp, dt in [("d_partials", [128, 8], F32),
                            ("d_hmL", [128, 400], F32),
                            ("d_sc", [128, 16], F32),
                            ("d_gb", [128, 4], F32),
                            ("d_gbn", [128, 4], F32),
                            ("d_tbox", [128, 4], F32)]:
            dbg[nm] = nc.declare_dram_parameter(nm, shp, dt, isOutput=True)

    es = ExitStack()
    dS = es.enter_context(nc.semaphore("dS"))        # small input dmas
    dH = es.enter_context(nc.semaphore("dH"))        # hm2
    dQ = es.enter_context(nc.semaphore("dQ"))        # qgrid
    dW = es.enter_context(nc.semaphore("dW"))        # wt128
    dO = es.enter_context(nc.semaphore("dO"))        # output
    va = es.enter_context(nc.semaphore("va"))        # vector -> scalar
    av = es.enter_context(nc.semaphore("av"))        # scalar -> vector
    g_s = es.enter_context(nc.semaphore("g_s"))      # gathers done
    pe_s = es.enter_context(nc.semaphore("pe_s"))    # matmul per group
    dv_s = es.enter_context(nc.semaphore("dv_s"))    # slab reduce done
    pt_s = es.enter_context(nc.semaphore("pt_s"))    # partials ready
    pv_s = es.enter_context(nc.semaphore("pv_s"))    # pvec ready

    sQ = es.enter_context(nc.sbuf_tensor("sQ", [128, NGRP * 128], BF16))
    wt128 = es.enter_context(nc.sbuf_tensor("wt128", [128, 512], BF16))
    cvec = es.enter_context(nc.sbuf_tensor("cvec_s", [128, 8], F32))
    sci = es.enter_context(nc.sbuf_tensor("sci", [128, 1], I32))
    hmP = es.enter_context(nc.sbuf_tensor("hmP", [128, 400], F32))
    lnp = es.enter_context(nc.sbuf_tensor("lnp", [128, 400], F32))
    ln1p = es.enter_context(nc.sbuf_tensor("ln1p", [128, 400], F32))
    p2 = es.enter_context(nc.sbuf_tensor("p2", [128, 400], F32))
    q2 = es.enter_context(nc.sbuf_tensor("q2", [128, 400], F32))
    texp = es.enter_context(nc.sbuf_tensor("texp", [128, 400], F32))
    keepm = es.enter_context(nc.sbuf_tensor("keepm", [128, 400], F32))
    hmL = es.enter_context(nc.sbuf_tensor("hmL", [128, 400], F32))
    tbox = es.enter_context(nc.sbuf_tensor("tbox", [128, 4], F32))
    gb = es.enter_context(nc.sbuf_tensor("gb", [128, 4], F32))
    gbn = es.enter_context(nc.sbuf_tensor("gbn", [128, 4], F32))
    gc = es.enter_context(nc.sbuf_tensor("gc", [128, NCLS], F32))
    gcp = es.enter_context(nc.sbuf_tensor("gcp", [128, NCLS], F32))
    junk43 = es.enter_context(nc.sbuf_tensor("junk43", [128, NCLS], F32))
    jb43 = es.enter_context(nc.sbuf_tensor("jb43", [128, NCLS], F32))
    onehot = es.enter_context(nc.sbuf_tensor("onehot_s", [128, NCLS], F32))
    parti = es.enter_context(nc.sbuf_tensor("parti_s", [128, 8], F32))
    sc = es.enter_context(nc.sbuf_tensor("sc", [128, 16], F32))
    pv = es.enter_context(nc.sbuf_tensor("pv", [1, 8], F32))

    psA = es.enter_context(nc.psum_tensor("psA", [128, 2048], F32))
    psB = es.enter_context(nc.psum_tensor("psB", [128, 2048], F32))

    with es:
        names = ["l1r", "negrow", "plab", "lnpl", "ln1pl", "p2pl", "q2pl",
                 "cva", "sent"]
        col = {n: sc[:, i: i + 1] for i, n in enumerate(names)}
        nc.const_aps.aps[(F32, 0.0)] = cvec[:, 0:1]
        nc.const_aps.aps[(F32, 1.0)] = cvec[:, 1:2]
        kept = parti[:, 2:3]
        keep2 = parti[:, 5:6]
        cellg = sci[:, 0:1]

        def slab_ngroups(k):
            return min(4, NGRP - 4 * k)

        def slab_in(k):
            pst = psA if k % 2 == 0 else psB
            return pst[:, 0: 512 * slab_ngroups(k)]

        def slab_out(k):
            return hmL[:, 32 * k: 32 * k + 8 * slab_ngroups(k)]

        with nc.Block() as block:

            @block.sync
            def _(sync):
                sync.dma_start(out=parti[:, :], in_=pi_d[:, :]).then_inc(dS, 16)
                sync.dma_start(out=sci[:, :], in_=cg_d[:, :]).then_inc(dS, 16)
                sync.dma_start(out=tbox[:, :], in_=tb_d[:, :]).then_inc(dS, 16)
                sync.dma_start(out=onehot[:, :], in_=oh_d[:, :]).then_inc(dS, 16)
                sync.dma_start(out=cvec[:, :], in_=cv_d[:, :]).then_inc(dS, 16)
                sync.dma_start(out=hmP[:, :], in_=hm_d[:, :]).then_inc(dH, 16)
                sync.dma_start(out=wt128[:, :], in_=wt_d[:, :]).then_inc(dW, 16)
                sync.dma_start(out=sQ[:, :], in_=qg_d[:, :]).then_inc(dQ, 16)
                sync.wait_ge(pv_s, 1)
                sync.dma_start(out=out_d[:, :], in_=pv[:, :]).then_inc(dO, 16)
                ndO = 16
                if debug:
                    for nm, t in [("d_partials", parti), ("d_hmL", hmL),
                                  ("d_sc", sc), ("d_gb", gb),
                                  ("d_gbn", gbn), ("d_tbox", tbox)]:
                        sync.dma_start(out=dbg[nm][:, :], in_=t[:, :]).then_inc(dO, 16)
                        ndO += 16
                sync.wait_ge(dO, ndO)

            @block.gpsimd
            def _(gpsimd):
                gpsimd.wait_ge(dS, 80)
                gpsimd.indirect_dma_start(
                    out=gb[:, :], out_offset=None,
                    in_=pb_d[:, :],
                    in_offset=bass.IndirectOffsetOnAxis(ap=cellg, axis=0),
                ).then_inc(g_s, 16)
                gpsimd.indirect_dma_start(
                    out=gc[:, :], out_offset=None,
                    in_=pc_d[:, :],
                    in_offset=bass.IndirectOffsetOnAxis(ap=cellg, axis=0),
                ).then_inc(g_s, 16)
                # partition-sum of partials replaces the PE ones-matmul
                gpsimd.wait_ge(pt_s, 1)
                gpsimd.tensor_reduce(out=pv[0:1, :], in_=parti[:, :],
                                     op=ALU.add, axis=mybir.AxisListType.C)
                gpsimd.drain().then_inc(pv_s, 1)

            @block.tensor
            def _(tensor):
                tensor.wait_ge(dW, 16)
                tensor.wait_ge(dQ, 16)
                for g in range(NGRP):
                    pst = psA if (g // 4) % 2 == 0 else psB
                    if g >= 8:
                        tensor.wait_ge(dv_s, g // 4 - 1)
                    tensor.matmul(
                        pst[:, 512 * (g % 4): 512 * (g % 4 + 1)],
                        sQ[:, g * 128: (g + 1) * 128],
                        wt128[:, :],
                        start=True, stop=True, skip_group_check=True,
                    ).then_inc(pe_s, 1)

            @block.scalar
            def _(scalar):
                # pred-heatmap transcendentals (read clipped hmP only)
                scalar.wait_ge(va, 1)
                scalar.activation(lnp[:, :], hmP[:, :], ACT.Ln)
                scalar.activation(ln1p[:, :], hmP[:, :], ACT.Ln, bias=1.0, scale=-1.0)
                scalar.activation(p2[:, :], hmP[:, :], ACT.Square)
                scalar.activation(q2[:, :], hmP[:, :], ACT.Square, bias=1.0, scale=-1.0)
                scalar.drain().then_inc(av, 1)                      # av1
                # cls sigmoid numerator
                scalar.wait_ge(g_s, 32)
                scalar.activation(gcp[:, :], gc[:, :], ACT.Exp, scale=-1.0)
                scalar.drain().then_inc(av, 1)                      # av2
                scalar.wait_ge(va, 2)
                scalar.activation(junk43[:, :], gcp[:, :], ACT.Ln, bias=1.0, scale=-1.0)
                scalar.activation(gc[:, :], gcp[:, :], ACT.Square)
                scalar.drain().then_inc(av, 1)                      # av3
                scalar.wait_ge(va, 3)
                scalar.activation(col["lnpl"], col["plab"], ACT.Ln)
                scalar.activation(col["ln1pl"], col["plab"], ACT.Ln, bias=1.0, scale=-1.0)
                scalar.activation(col["p2pl"], col["plab"], ACT.Square)
                scalar.activation(col["q2pl"], col["plab"], ACT.Square, bias=1.0, scale=-1.0)
                scalar.drain().then_inc(av, 1)                      # av4
                # per-slab exp of the log-heatmap target
                for k in range(NSLAB):
                    scalar.wait_ge(dv_s, min(k + 2, NSLAB))
                    n8 = 8 * slab_ngroups(k)
                    scalar.activation(texp[:, 32 * k: 32 * k + n8],
                                      hmL[:, 32 * k: 32 * k + n8], ACT.Exp)
                scalar.drain().then_inc(av, 1)                      # av5

            @block.vector
            def _(v):
                ts, stt = v.tensor_scalar, v.scalar_tensor_tensor

                def slot(*thunks):
                    for t in thunks:
                        t()
                    v.drain()

                # clip predicted heatmap
                v.wait_ge(dH, 16)
                slot(lambda: ts(hmP[:, :], hmP[:, :], EPS, 1.0 - EPS, op0=ALU.max, op1=ALU.min))
                v.sem_inc(va, 1)                                    # va1
                # heatmap slab reduces + interleaved one-shot work
                for k in range(NSLAB):
                    v.wait_ge(pe_s, min(4 * k + 4, NGRP))
                    v.tensor_reduce(
                        out=slab_out(k),
                        in_=slab_in(k).rearrange("p (G m) -> p G m",
                                                 G=8 * slab_ngroups(k)),
                        op=ALU.max, axis=AXX,
                    ).then_inc(dv_s, 1)
                    if k == 1:
                        # box l1
                        v.wait_ge(g_s, 32)
                        slot(lambda: v.tensor_sub(gbn[:, :], gb[:, :], tbox[:, :]))
                        slot(lambda: ts(gb[:, :], gbn[:, :], -1.0, None, op0=ALU.mult))
                        slot(lambda: v.tensor_tensor(gbn[:, :], gbn[:, :], gb[:, :], op=ALU.max))
                        slot(lambda: v.tensor_reduce(out=col["l1r"], in_=gbn[:, :], op=ALU.add, axis=AXX))
                        slot(lambda: v.tensor_mul(parti[:, 3:4], col["l1r"], kept))
                    if k == 2:
                        v.wait_ge(av, 2)
                        slot(lambda: ts(gcp[:, :], gcp[:, :], 1.0, None, op0=ALU.add))
                        slot(lambda: v.reciprocal(gcp[:, :], gcp[:, :]))
                        slot(lambda: ts(gcp[:, :], gcp[:, :], EPS, 1.0 - EPS, op0=ALU.max, op1=ALU.min))
                        v.sem_inc(va, 1)                            # va2
                    if k == 5:
                        v.wait_ge(av, 3)
                        slot(lambda: stt(junk43[:, :], gc[:, :], -0.75, junk43[:, :],
                                         op0=ALU.mult, op1=ALU.mult, accum_out=col["negrow"]),
                             lambda: v.tensor_mul(jb43[:, :], gcp[:, :], onehot[:, :]))
                        slot(lambda: v.tensor_mul(parti[:, 4:5], col["negrow"], kept),
                             lambda: v.tensor_reduce(out=col["plab"], in_=jb43[:, :], op=ALU.add, axis=AXX))
                        v.sem_inc(va, 1)                            # va3
                    if k == 8:
                        v.wait_ge(av, 4)
                        slot(lambda: stt(col["cva"], col["q2pl"], -0.25, col["lnpl"], op0=ALU.mult, op1=ALU.mult),
                             lambda: stt(col["sent"], col["p2pl"], -0.75, col["ln1pl"], op0=ALU.mult, op1=ALU.mult))
                        slot(lambda: v.tensor_sub(col["cva"], col["cva"], col["sent"]))
                        slot(lambda: v.tensor_mul(parti[:, 6:7], col["cva"], keep2))
                # ---- heat focal tail ----
                v.drain()
                slot(lambda: ts(keepm[:, :], hmL[:, :], THR, None, op0=ALU.is_ge))
                v.wait_ge(av, 5)
                slot(lambda: v.tensor_mul(texp[:, :], texp[:, :], keepm[:, :]))
                slot(lambda: stt(lnp[:, :], q2[:, :], -0.25, lnp[:, :], op0=ALU.mult, op1=ALU.mult),
                     lambda: stt(ln1p[:, :], p2[:, :], 0.75, ln1p[:, :], op0=ALU.mult, op1=ALU.mult))
                slot(lambda: v.tensor_mul(lnp[:, :], lnp[:, :], texp[:, :]),
                     lambda: stt(ln1p[:, :], texp[:, :], 1.0, ln1p[:, :], op0=ALU.subtract, op1=ALU.mult))
                slot(lambda: ts(q2[:, :], texp[:, :], 0.5, None, op0=ALU.is_gt))
                slot(lambda: v.tensor_reduce(out=parti[:, 0:1], in_=q2[:, :], op=ALU.add, axis=AXX),
                     lambda: v.tensor_sub(lnp[:, :], lnp[:, :], ln1p[:, :]))
                slot(lambda: v.tensor_mul(lnp[:, :], lnp[:, :], q2[:, :]))
                slot(lambda: stt(lnp[:, :], lnp[:, :], 1.0, ln1p[:, :],
                                 op0=ALU.mult, op1=ALU.add, accum_out=parti[:, 1:2]))
                v.sem_inc(pt_s, 1)

    return nc


_CACHE = {}


def _consts():
    j = np.arange(PIX)
    x = (j % W).astype(np.float32)
    y = (j // W).astype(np.float32)
    q1 = x * x + y * y
    q1hi = q1.astype(ml_dtypes.bfloat16).astype(np.float32)
    q1lo = q1 - q1hi
    onesv = np.ones_like(x)
    q5 = np.stack([q1hi, q1lo, x, y, onesv])           # [5, PIX] all bf16-exact
    # qg[32 c + 5 s + r, 128 g + p] = q5[r, 512 g + 128 c + p]
    q5r = q5.reshape(5, NGRP, 4, 128)                  # [r, g, c, p]
    qg = np.zeros((128, NGRP * 128), np.float32)
    for c in range(4):
        for s in range(3):
            qg[32 * c + 5 * s: 32 * c + 5 * s + 5, :] = (
                q5r[:, :, c, :].reshape(5, NGRP * 128))
    qg = qg.astype(ml_dtypes.bfloat16)
    cvec = np.zeros((128, 8), dtype=np.float32)
    cvec[:, 1] = 1.0
    return qg, cvec


def _last_wins_kept(keys, valid):
    """kept[i] = valid[i] and no valid j>i with keys[j]==keys[i]."""
    n = len(keys)
    kept = np.zeros(n, bool)
    seen = set()
    for i in range(n - 1, -1, -1):
        if valid[i] and keys[i] not in seen:
            kept[i] = True
            seen.add(keys[i])
    return kept


def _stage_a(bboxes, labels):
    """Per-core-chunk object prep: returns wt128, cellg, tbox, onehot, parti."""
    f32 = np.float32
    bb = bboxes.reshape(128, 4).astype(f32)
    lab = labels.reshape(128).astype(np.int64)
    x1, y1, x2, y2 = bb[:, 0], bb[:, 1], bb[:, 2], bb[:, 3]
    cx = (x1 + x2) / f32(2.0)
    cy = (y1 + y2) / f32(2.0)
    bw = x2 - x1
    bh = y2 - y1
    valid = (lab >= 0) & (bb.sum(1) > 0) & (bw > 0) & (bh > 0)
    gx = np.clip((cx / f32(4.0)).astype(np.int32), 0, W - 1)
    gy = np.clip((cy / f32(4.0)).astype(np.int32), 0, H - 1)
    r = np.maximum(np.sqrt(np.maximum(bw * bh, f32(0.0))) / f32(4.0), f32(2.0)).astype(np.int32).astype(f32)
    nscv = f32(-2.0) / (r * r)
    gxf = gx.astype(f32)
    gyf = gy.astype(f32)
    w1 = np.where(valid, nscv, f32(0))
    w2 = np.where(valid, f32(-2.0) * nscv * gxf, f32(0))
    w3 = np.where(valid, f32(-2.0) * nscv * gyf, f32(0))
    w4 = np.where(valid, nscv * (gxf * gxf + gyf * gyf), f32(-1e30))
    Wm = np.stack([w1, w1, w2, w3, w4]).astype(f32)    # [5, 128]
    # 3-way bf16 split
    a_ = Wm.astype(ml_dtypes.bfloat16).astype(f32)
    r1_ = Wm - a_
    b_ = r1_.astype(ml_dtypes.bfloat16).astype(f32)
    r2_ = r1_ - b_
    c_ = r2_.astype(ml_dtypes.bfloat16).astype(f32)
    w15 = np.concatenate([a_, b_, c_], axis=0)         # [15, 128]
    wt128 = np.zeros((128, 512), np.float32)
    for c in range(4):
        wt128[32 * c: 32 * c + 15, 128 * c: 128 * (c + 1)] = w15
    wt128 = wt128.astype(ml_dtypes.bfloat16)

    img = np.arange(128) // M
    cell = gy.astype(np.int64) * W + gx.astype(np.int64)
    cellg = (cell + img * PIX).astype(np.int32)
    kept = _last_wins_kept(list(cellg), valid)
    labc = np.clip(lab, 0, NCLS - 1)
    key2 = cellg.astype(np.int64) * NCLS + labc
    keep2 = _last_wins_kept(list(key2), valid)

    tbox = np.zeros((128, 4), np.float32)
    tbox[:, 0] = np.where(valid, cx / f32(4.0) - gxf - f32(0.5), f32(0.0))
    tbox[:, 1] = np.where(valid, cy / f32(4.0) - gyf - f32(0.5), f32(0.0))
    tbox[:, 2] = np.where(valid, np.log(np.maximum(bw * f32(0.25) + f32(1e-6), f32(1e-20))).astype(f32), f32(0.0))
    tbox[:, 3] = np.where(valid, np.log(np.maximum(bh * f32(0.25) + f32(1e-6), f32(1e-20))).astype(f32), f32(0.0))
    onehot = (labc[:, None] == np.arange(NCLS)[None, :]).astype(np.float32)
    parti = np.zeros((128, 8), np.float32)
    parti[:, 2] = kept.astype(np.float32)
    parti[:, 5] = keep2.astype(np.float32)
    return wt128, cellg.reshape(128, 1), tbox, onehot, parti


def _prep(pred_heatmap, pred_boxes, pred_classes, bboxes, labels):
    qg, cvec = _consts()
    pbt = np.ascontiguousarray(
        pred_boxes.transpose(0, 2, 3, 1).reshape(B, PIX, 4)).astype(np.float32)
    pct = np.ascontiguousarray(
        pred_classes.transpose(0, 2, 3, 1).reshape(B, PIX, NCLS)).astype(np.float32)
    # hm2[p, 2 f + img] = hm[img, 128 f + p]
    hmr = np.asarray(pred_heatmap, np.float32).reshape(B, 200, 128)
    bbn = np.asarray(bboxes, np.float32)
    labn = np.asarray(labels)
    in_maps = []
    for c in range(NC):
        s = slice(c * BPC, (c + 1) * BPC)
        hm2 = np.ascontiguousarray(hmr[s].transpose(2, 1, 0).reshape(128, 400))
        wt128, cellg, tbox, onehot, parti = _stage_a(bbn[s], labn[s])
        in_maps.append({
            "qg": qg,
            "wt": wt128,
            "hm2": hm2,
            "pbt": pbt[s].reshape(BPC * PIX, 4),
            "pct": pct[s].reshape(BPC * PIX, NCLS),
            "cellg": cellg, "tboxd": tbox, "onehot": onehot,
            "parti": parti, "cvec": cvec,
        })
    return in_maps


def _combine(pvecs):
    P = np.sum(np.stack(pvecs, 0), axis=0, dtype=np.float64).astype(np.float32)
    heat = P[1] / max(P[0], np.float32(1.0))
    num_pos = max(P[2], np.float32(1.0))
    box = P[3] / num_pos if P[2] > 1.0 else np.float32(0.0)
    cls = (P[4] + P[6]) / max(P[5], np.float32(1.0)) if P[2] > 1.0 else np.float32(0.0)
    return np.float32(heat + box + cls)


def kernel(pred_heatmap, pred_boxes, pred_classes, bboxes, labels):
    if "nc" not in _CACHE:
        _CACHE["nc"] = _build()
    nc = _CACHE["nc"]
    in_maps = _prep(pred_heatmap, pred_boxes, pred_classes, bboxes, labels)
    r = run_bass_kernel_spmd(nc, in_maps, list(range(NC)))
    pvecs = [np.asarray(r.results[c]["out"]).reshape(8) for c in range(NC)]
    return _combine(pvecs)


if __name__ == "__main__":
    import reference
    inputs = reference.setup_inputs()
    inputs = {k: np.asarray(v) for k, v in inputs.items()}
    out = kernel(**inputs)
    exp = np.asarray(reference.reference(**inputs))
    rel = abs(out - exp) / max(abs(exp), 1e-9)
    print("expected:", exp, "actual:", out, "rel:", rel)


# revision 9
# speedup vs baseline: 3.2532x; 1.2636x over previous
"""AnchorFreeLoss on 8 TRN2 NeuronCores (v3).

Strategy (data-parallel over batch, 2 images/core):
- Host prep (tiny [B,M] per-object math, like the qgrid/NHWC transposes):
  per-object centers/radii -> packed block-diagonal bf16 coefficient
  matrix wt128 (3-way bf16 split of fp32 coefficients, quadrant-aligned),
  gather offsets, box targets, class one-hots, and scatter dedup flags
  (last valid object per cell wins, matching XLA scatter semantics).
- Device heatmap target: logG[pix, m] = -dist^2/(2*sigma^2) is affine in
  q(pix) = [x^2+y^2, x, y, 1]: ONE bf16 matmul per 512-pixel group
  (K=128 packs 4 pixel-chunks x 15 live rows) into two 4-bank PSUM slabs;
  DVE max-reduces a whole 2048-col slab per instruction. Cutoff dist<=2r
  equals logG >= -8 exactly (sigma = r/2).
- Focal transcendentals on the Scalar engine (incl. per-slab exp of the
  log-heatmap); focal algebra + reductions on DVE.
- Box/class losses only touch object-center cells: GPSIMD indirect-DMA
  gathers from host-transposed [B*H*W, C] tables.
- No collective: GPSIMD cross-lane-reduces the per-partition partial
  sums; each core DMAs out 8 floats; host combines (the unshard step).
"""

import sys
from contextlib import ExitStack

import numpy as np

if "/opt/trn_rl_repo" not in sys.path:
    sys.path.insert(0, "/opt/trn_rl_repo")

import ml_dtypes
from concourse import bass, mybir
from concourse.bass_utils import run_bass_kernel_spmd

F32 = mybir.dt.float32
BF16 = mybir.dt.bfloat16
I32 = mybir.dt.int32
ALU = mybir.AluOpType
ACT = mybir.ActivationFunctionType
AXX = mybir.AxisListType.X

B, M, H, W = 16, 64, 160, 160
NC = 8
BPC = B // NC          # 2 images per core
PIX = H * W            # 25600
NCLS = 43
EPS = 1e-7
THR = -8.0             # log-domain cutoff (= dist <= 2r since sigma = r/2)
NGRP = PIX // 512      # 50 groups of 512 pixels
NSLAB = (NGRP + 3) // 4  # 13 slabs (last one half-size)


def _build(debug=False):
    nc = bass.Bass()

    qg_d = nc.declare_dram_parameter("qg", [60, NGRP * 128], BF16, isOutput=False)
    wt_d = nc.declare_dram_parameter("wt", [60, 512], BF16, isOutput=False)
    hm_d = nc.declare_dram_parameter("hm2", [128, 400], F32, isOutput=False)
    pb_d = nc.declare_dram_parameter("pbt", [BPC * PIX, 4], F32, isOutput=False)
    pc_d = nc.declare_dram_parameter("pct", [BPC * PIX, NCLS], F32, isOutput=False)
    cg_d = nc.declare_dram_parameter("cellg", [128, 1], I32, isOutput=False)
    tb_d = nc.declare_dram_parameter("tboxd", [128, 4], F32, isOutput=False)
    oh_d = nc.declare_dram_parameter("onehot", [128, NCLS], F32, isOutput=False)
    pi_d = nc.declare_dram_parameter("parti", [128, 8], F32, isOutput=False)
    cv_d = nc.declare_dram_parameter("cvec", [128, 8], F32, isOutput=False)
    out_d = nc.declare_dram_parameter("out", [1, 8], F32, isOutput=True)
    dbg = {}
    if debug:
        for nm, shp, dt in [("d_partials", [128, 8], F32),
                            ("d_hmL", [128, 400], F32),
                            ("d_sc", [128, 16], F32),
                            ("d_gb", [128, 4], F32),
                            ("d_gbn", [128, 4], F32),
                            ("d_tbox", [128, 4], F32)]:
            dbg[nm] = nc.declare_dram_parameter(nm, shp, dt, isOutput=True)

    es = ExitStack()
    dS = es.enter_context(nc.semaphore("dS"))        # small input dmas
    dH = es.enter_context(nc.semaphore("dH"))        # hm2
    dQ = es.enter_context(nc.semaphore("dQ"))        # qgrid
    dW = es.enter_context(nc.semaphore("dW"))        # wt128
    dO = es.enter_context(nc.semaphore("dO"))        # output
    va = es.enter_context(nc.semaphore("va"))        # vector -> scalar
    av = es.enter_context(nc.semaphore("av"))        # scalar -> vector
    g_s = es.enter_context(nc.semaphore("g_s"))      # gathers done
    pe_s = es.enter_context(nc.semaphore("pe_s"))    # matmul per group
    dv_s = es.enter_context(nc.semaphore("dv_s"))    # slab reduce done
    pt_s = es.enter_context(nc.semaphore("pt_s"))    # partials ready
    pp_s = es.enter_context(nc.semaphore("pp_s"))    # psp matmul done
    pv_s = es.enter_context(nc.semaphore("pv_s"))    # pvec ready

    sQ = es.enter_context(nc.sbuf_tensor("sQ", [60, NGRP * 128], BF16))
    wt128 = es.enter_context(nc.sbuf_tensor("wt128", [60, 512], BF16))
    cvec = es.enter_context(nc.sbuf_tensor("cvec_s", [128, 8], F32))
    sci = es.enter_context(nc.sbuf_tensor("sci", [128, 1], I32))
    hmP = es.enter_context(nc.sbuf_tensor("hmP", [128, 400], F32))
    lnp = es.enter_context(nc.sbuf_tensor("lnp", [128, 400], F32))
    ln1p = es.enter_context(nc.sbuf_tensor("ln1p", [128, 400], F32))
    p2 = es.enter_context(nc.sbuf_tensor("p2", [128, 400], F32))
    q2 = es.enter_context(nc.sbuf_tensor("q2", [128, 400], F32))
    texp = es.enter_context(nc.sbuf_tensor("texp", [128, 400], F32))
    hmL = es.enter_context(nc.sbuf_tensor("hmL", [128, 400], F32))
    tbox = es.enter_context(nc.sbuf_tensor("tbox", [128, 4], F32))
    gb = es.enter_context(nc.sbuf_tensor("gb", [128, 4], F32))
    gbn = es.enter_context(nc.sbuf_tensor("gbn", [128, 4], F32))
    gc = es.enter_context(nc.sbuf_tensor("gc", [128, NCLS], F32))
    gcp = es.enter_context(nc.sbuf_tensor("gcp", [128, NCLS], F32))
    junk43 = es.enter_context(nc.sbuf_tensor("junk43", [128, NCLS], F32))
    jb43 = es.enter_context(nc.sbuf_tensor("jb43", [128, NCLS], F32))
    onehot = es.enter_context(nc.sbuf_tensor("onehot_s", [128, NCLS], F32))
    parti = es.enter_context(nc.sbuf_tensor("parti_s", [128, 8], F32))
    sc = es.enter_context(nc.sbuf_tensor("sc", [128, 16], F32))
    pv = es.enter_context(nc.sbuf_tensor("pv", [1, 8], F32))

    psA = es.enter_context(nc.psum_tensor("psA", [128, 2048], F32))
    psB = es.enter_context(nc.psum_tensor("psB", [128, 2048], F32))

    with es:
        names = ["l1r", "negrow", "plab", "lnpl", "ln1pl", "p2pl", "q2pl",
                 "cva", "sent"]
        col = {n: sc[:, i: i + 1] for i, n in enumerate(names)}
        nc.const_aps.aps[(F32, 0.0)] = cvec[:, 0:1]
        nc.const_aps.aps[(F32, 1.0)] = cvec[:, 1:2]
        ones = cvec[:, 1:2]
        kept = parti[:, 2:3]
        keep2 = parti[:, 5:6]
        cellg = sci[:, 0:1]

        def slab_ngroups(k):
            return min(4, NGRP - 4 * k)

        def slab_in(k):
            pst = psA if k % 2 == 0 else psB
            return pst[:, 0: 512 * slab_ngroups(k)]

        def slab_out(k):
            return hmL[:, 32 * k: 32 * k + 8 * slab_ngroups(k)]

        with nc.Block() as block:

            @block.sync
            def _(sync):
                sync.dma_start(out=sQ[:, :], in_=qg_d[:, :]).then_inc(dQ, 16)
                sync.dma_start(out=parti[:, :], in_=pi_d[:, :]).then_inc(dS, 16)
                sync.dma_start(out=sci[:, :], in_=cg_d[:, :]).then_inc(dS, 16)
                sync.dma_start(out=tbox[:, :], in_=tb_d[:, :]).then_inc(dS, 16)
                sync.dma_start(out=onehot[:, :], in_=oh_d[:, :]).then_inc(dS, 16)
                sync.dma_start(out=cvec[:, :], in_=cv_d[:, :]).then_inc(dS, 16)
                sync.dma_start(out=hmP[:, :], in_=hm_d[:, :]).then_inc(dH, 16)
                sync.dma_start(out=wt128[:, :], in_=wt_d[:, :]).then_inc(dW, 16)
                sync.wait_ge(pv_s, 1)
                sync.dma_start(out=out_d[:, :], in_=pv[:, :]).then_inc(dO, 16)
                ndO = 16
                if debug:
                    for nm, t in [("d_partials", parti), ("d_hmL", hmL),
                                  ("d_sc", sc), ("d_gb", gb),
                                  ("d_gbn", gbn), ("d_tbox", tbox)]:
                        sync.dma_start(out=dbg[nm][:, :], in_=t[:, :]).then_inc(dO, 16)
                        ndO += 16
                sync.wait_ge(dO, ndO)

            @block.gpsimd
            def _(gpsimd):
                gpsimd.wait_ge(dS, 80)
                gpsimd.indirect_dma_start(
                    out=gb[:, :], out_offset=None,
                    in_=pb_d[:, :],
                    in_offset=bass.IndirectOffsetOnAxis(ap=cellg, axis=0),
                ).then_inc(g_s, 16)
                gpsimd.indirect_dma_start(
                    out=gc[:, :], out_offset=None,
                    in_=pc_d[:, :],
                    in_offset=bass.IndirectOffsetOnAxis(ap=cellg, axis=0),
                ).then_inc(g_s, 16)


            @block.tensor
            def _(tensor):
                tensor.wait_ge(dW, 16)
                tensor.wait_ge(dQ, 16)
                for g in range(NGRP):
                    pst = psA if (g // 4) % 2 == 0 else psB
                    if g >= 8:
                        tensor.wait_ge(dv_s, g // 4 - 1)
                    tensor.matmul(
                        pst[:, 512 * (g % 4): 512 * (g % 4 + 1)],
                        sQ[:, g * 128: (g + 1) * 128],
                        wt128[:, :],
                        start=True, stop=True, skip_group_check=True,
                    ).then_inc(pe_s, 1)
                # partition-sum of partials via ones-matmul into a psA corner
                tensor.wait_ge(pt_s, 1)
                tensor.matmul(psA[0:1, 0:8], ones, parti[:, :], start=True,
                              stop=True, skip_group_check=True).then_inc(pp_s, 1)

            @block.scalar
            def _(scalar):
                # pred-heatmap transcendentals (read clipped hmP only)
                scalar.wait_ge(va, 1)
                scalar.activation(lnp[:, :], hmP[:, :], ACT.Ln)
                scalar.activation(ln1p[:, :], hmP[:, :], ACT.Ln, bias=1.0, scale=-1.0)
                scalar.activation(p2[:, :], hmP[:, :], ACT.Square)
                scalar.activation(q2[:, :], hmP[:, :], ACT.Square, bias=1.0, scale=-1.0)
                scalar.drain().then_inc(av, 1)                      # av1
                # cls sigmoid numerator
                scalar.wait_ge(g_s, 32)
                scalar.activation(gcp[:, :], gc[:, :], ACT.Exp, scale=-1.0)
                scalar.drain().then_inc(av, 1)                      # av2
                scalar.wait_ge(va, 2)
                scalar.activation(junk43[:, :], gcp[:, :], ACT.Ln, bias=1.0, scale=-1.0)
                scalar.activation(gc[:, :], gcp[:, :], ACT.Square)
                scalar.drain().then_inc(av, 1)                      # av3
                scalar.wait_ge(va, 3)
                scalar.activation(col["lnpl"], col["plab"], ACT.Ln)
                scalar.activation(col["ln1pl"], col["plab"], ACT.Ln, bias=1.0, scale=-1.0)
                scalar.activation(col["p2pl"], col["plab"], ACT.Square)
                scalar.activation(col["q2pl"], col["plab"], ACT.Square, bias=1.0, scale=-1.0)
                scalar.drain().then_inc(av, 1)                      # av4
                # per-slab exp of the log-heatmap target
                for k in range(NSLAB):
                    scalar.wait_ge(dv_s, min(k + 2, NSLAB))
                    n8 = 8 * slab_ngroups(k)
                    scalar.activation(texp[:, 32 * k: 32 * k + n8],
                                      hmL[:, 32 * k: 32 * k + n8], ACT.Exp)
                scalar.drain().then_inc(av, 1)                      # av5

            @block.vector
            def _(v):
                ts, stt = v.tensor_scalar, v.scalar_tensor_tensor

                def slot(*thunks):
                    for t in thunks:
                        t()
                    v.drain()

                # clip predicted heatmap
                v.wait_ge(dH, 16)
                slot(lambda: ts(hmP[:, :], hmP[:, :], EPS, 1.0 - EPS, op0=ALU.max, op1=ALU.min))
                v.sem_inc(va, 1)                                    # va1
                # heatmap slab reduces + interleaved one-shot work
                for k in range(NSLAB):
                    v.wait_ge(pe_s, min(4 * k + 4, NGRP))
                    v.tensor_reduce(
                        out=slab_out(k),
                        in_=slab_in(k).rearrange("p (G m) -> p G m",
                                                 G=8 * slab_ngroups(k)),
                        op=ALU.max, axis=AXX,
                    ).then_inc(dv_s, 1)
                    if k == 1:
                        # box l1
                        v.wait_ge(g_s, 32)
                        slot(lambda: v.tensor_sub(gbn[:, :], gb[:, :], tbox[:, :]))
                        slot(lambda: ts(gb[:, :], gbn[:, :], -1.0, None, op0=ALU.mult))
                        slot(lambda: v.tensor_tensor(gbn[:, :], gbn[:, :], gb[:, :], op=ALU.max))
                        slot(lambda: v.tensor_reduce(out=col["l1r"], in_=gbn[:, :], op=ALU.add, axis=AXX))
                        slot(lambda: v.tensor_mul(parti[:, 3:4], col["l1r"], kept))
                    if k == 2:
                        v.wait_ge(av, 2)
                        slot(lambda: ts(gcp[:, :], gcp[:, :], 1.0, None, op0=ALU.add))
                        slot(lambda: v.reciprocal(gcp[:, :], gcp[:, :]))
                        slot(lambda: ts(gcp[:, :], gcp[:, :], EPS, 1.0 - EPS, op0=ALU.max, op1=ALU.min))
                        v.sem_inc(va, 1)                            # va2
                    if k == 3:
                        # focal A/B coefficient maps (need only scalar av1)
                        slot(lambda: stt(lnp[:, :], q2[:, :], -0.25, lnp[:, :], op0=ALU.mult, op1=ALU.mult),
                             lambda: stt(ln1p[:, :], p2[:, :], 0.75, ln1p[:, :], op0=ALU.mult, op1=ALU.mult))
                    if k == 5:
                        v.wait_ge(av, 3)
                        slot(lambda: stt(junk43[:, :], gc[:, :], -0.75, junk43[:, :],
                                         op0=ALU.mult, op1=ALU.mult, accum_out=col["negrow"]),
                             lambda: v.tensor_mul(jb43[:, :], gcp[:, :], onehot[:, :]))
                        slot(lambda: v.tensor_mul(parti[:, 4:5], col["negrow"], kept),
                             lambda: v.tensor_reduce(out=col["plab"], in_=jb43[:, :], op=ALU.add, axis=AXX))
                        v.sem_inc(va, 1)                            # va3
                    if k == 8:
                        v.wait_ge(av, 4)
                        slot(lambda: stt(col["cva"], col["q2pl"], -0.25, col["lnpl"], op0=ALU.mult, op1=ALU.mult),
                             lambda: stt(col["sent"], col["p2pl"], -0.75, col["ln1pl"], op0=ALU.mult, op1=ALU.mult))
                        slot(lambda: v.tensor_sub(col["cva"], col["cva"], col["sent"]))
                        slot(lambda: v.tensor_mul(parti[:, 6:7], col["cva"], keep2))
                # ---- heat focal tail (cutoff mask dropped: exp(logG) <= e^-8
                # below threshold, which perturbs only (1-t) by <=3.4e-4) ----
                v.drain()
                v.wait_ge(av, 5)
                slot(lambda: v.tensor_mul(lnp[:, :], lnp[:, :], texp[:, :]),
                     lambda: stt(ln1p[:, :], texp[:, :], 1.0, ln1p[:, :], op0=ALU.subtract, op1=ALU.mult))
                slot(lambda: ts(q2[:, :], texp[:, :], 0.5, None, op0=ALU.is_gt))
                slot(lambda: v.tensor_reduce(out=parti[:, 0:1], in_=q2[:, :], op=ALU.add, axis=AXX),
                     lambda: v.tensor_sub(lnp[:, :], lnp[:, :], ln1p[:, :]))
                slot(lambda: v.tensor_mul(lnp[:, :], lnp[:, :], q2[:, :]))
                slot(lambda: stt(lnp[:, :], lnp[:, :], 1.0, ln1p[:, :],
                                 op0=ALU.mult, op1=ALU.add, accum_out=parti[:, 1:2]))
                v.sem_inc(pt_s, 1)
                # final partial vector out
                v.wait_ge(pp_s, 1)
                slot(lambda: v.tensor_copy(pv[:, :], psA[0:1, 0:8]))
                v.sem_inc(pv_s, 1)

    return nc


_CACHE = {}


def _consts():
    j = np.arange(PIX)
    x = (j % W).astype(np.float32)
    y = (j // W).astype(np.float32)
    q1 = x * x + y * y
    q1hi = q1.astype(ml_dtypes.bfloat16).astype(np.float32)
    q1lo = q1 - q1hi
    onesv = np.ones_like(x)
    q5 = np.stack([q1hi, q1lo, x, y, onesv])           # [5, PIX] all bf16-exact
    # qg[15 c + 5 s + r, 128 g + p] = q5[r, 512 g + 128 c + p]
    q5r = q5.reshape(5, NGRP, 4, 128)                  # [r, g, c, p]
    qg = np.zeros((60, NGRP * 128), np.float32)
    for c in range(4):
        for s in range(3):
            qg[15 * c + 5 * s: 15 * c + 5 * s + 5, :] = (
                q5r[:, :, c, :].reshape(5, NGRP * 128))
    qg = qg.astype(ml_dtypes.bfloat16)
    cvec = np.zeros((128, 8), dtype=np.float32)
    cvec[:, 1] = 1.0
    return qg, cvec


def _last_wins_kept(keys, valid):
    """kept[i] = valid[i] and no valid j>i with keys[j]==keys[i]."""
    n = len(keys)
    kept = np.zeros(n, bool)
    seen = set()
    for i in range(n - 1, -1, -1):
        if valid[i] and keys[i] not in seen:
            kept[i] = True
            seen.add(keys[i])
    return kept


def _stage_a(bboxes, labels):
    """Per-core-chunk object prep: returns wt128, cellg, tbox, onehot, parti."""
    f32 = np.float32
    bb = bboxes.reshape(128, 4).astype(f32)
    lab = labels.reshape(128).astype(np.int64)
    x1, y1, x2, y2 = bb[:, 0], bb[:, 1], bb[:, 2], bb[:, 3]
    cx = (x1 + x2) / f32(2.0)
    cy = (y1 + y2) / f32(2.0)
    bw = x2 - x1
    bh = y2 - y1
    valid = (lab >= 0) & (bb.sum(1) > 0) & (bw > 0) & (bh > 0)
    gx = np.clip((cx / f32(4.0)).astype(np.int32), 0, W - 1)
    gy = np.clip((cy / f32(4.0)).astype(np.int32), 0, H - 1)
    r = np.maximum(np.sqrt(np.maximum(bw * bh, f32(0.0))) / f32(4.0), f32(2.0)).astype(np.int32).astype(f32)
    nscv = f32(-2.0) / (r * r)
    gxf = gx.astype(f32)
    gyf = gy.astype(f32)
    w1 = np.where(valid, nscv, f32(0))
    w2 = np.where(valid, f32(-2.0) * nscv * gxf, f32(0))
    w3 = np.where(valid, f32(-2.0) * nscv * gyf, f32(0))
    w4 = np.where(valid, nscv * (gxf * gxf + gyf * gyf), f32(-1e30))
    Wm = np.stack([w1, w1, w2, w3, w4]).astype(f32)    # [5, 128]
    # 3-way bf16 split
    a_ = Wm.astype(ml_dtypes.bfloat16).astype(f32)
    r1_ = Wm - a_
    b_ = r1_.astype(ml_dtypes.bfloat16).astype(f32)
    r2_ = r1_ - b_
    c_ = r2_.astype(ml_dtypes.bfloat16).astype(f32)
    w15 = np.concatenate([a_, b_, c_], axis=0)         # [15, 128]
    wt128 = np.zeros((60, 512), np.float32)
    for c in range(4):
        wt128[15 * c: 15 * c + 15, 128 * c: 128 * (c + 1)] = w15
    wt128 = wt128.astype(ml_dtypes.bfloat16)

    img = np.arange(128) // M
    cell = gy.astype(np.int64) * W + gx.astype(np.int64)
    cellg = (cell + img * PIX).astype(np.int32)
    kept = _last_wins_kept(list(cellg), valid)
    labc = np.clip(lab, 0, NCLS - 1)
    key2 = cellg.astype(np.int64) * NCLS + labc
    keep2 = _last_wins_kept(list(key2), valid)

    tbox = np.zeros((128, 4), np.float32)
    tbox[:, 0] = np.where(valid, cx / f32(4.0) - gxf - f32(0.5), f32(0.0))
    tbox[:, 1] = np.where(valid, cy / f32(4.0) - gyf - f32(0.5), f32(0.0))
    tbox[:, 2] = np.where(valid, np.log(np.maximum(bw * f32(0.25) + f32(1e-6), f32(1e-20))).astype(f32), f32(0.0))
    tbox[:, 3] = np.where(valid, np.log(np.maximum(bh * f32(0.25) + f32(1e-6), f32(1e-20))).astype(f32), f32(0.0))
    onehot = (labc[:, None] == np.arange(NCLS)[None, :]).astype(np.float32)
    parti = np.zeros((128, 8), np.float32)
    parti[:, 2] = kept.astype(np.float32)
    parti[:, 5] = keep2.astype(np.float32)
    return wt128, cellg.reshape(128, 1), tbox, onehot, parti


def _prep(pred_heatmap, pred_boxes, pred_classes, bboxes, labels):
    qg, cvec = _consts()
    pbt = np.ascontiguousarray(
        pred_boxes.transpose(0, 2, 3, 1).reshape(B, PIX, 4)).astype(np.float32)
    pct = np.ascontiguousarray(
        pred_classes.transpose(0, 2, 3, 1).reshape(B, PIX, NCLS)).astype(np.float32)
    # hm2[p, 2 f + img] = hm[img, 128 f + p]
    hmr = np.asarray(pred_heatmap, np.float32).reshape(B, 200, 128)
    bbn = np.asarray(bboxes, np.float32)
    labn = np.asarray(labels)
    in_maps = []
    for c in range(NC):
        s = slice(c * BPC, (c + 1) * BPC)
        hm2 = np.ascontiguousarray(hmr[s].transpose(2, 1, 0).reshape(128, 400))
        wt128, cellg, tbox, onehot, parti = _stage_a(bbn[s], labn[s])
        in_maps.append({
            "qg": qg,
            "wt": wt128,
            "hm2": hm2,
            "pbt": pbt[s].reshape(BPC * PIX, 4),
            "pct": pct[s].reshape(BPC * PIX, NCLS),
            "cellg": cellg, "tboxd": tbox, "onehot": onehot,
            "parti": parti, "cvec": cvec,
        })
    return in_maps


def _combine(pvecs):
    P = np.sum(np.stack(pvecs, 0), axis=0, dtype=np.float64).astype(np.float32)
    heat = P[1] / max(P[0], np.float32(1.0))
    num_pos = max(P[2], np.float32(1.0))
    box = P[3] / num_pos if P[2] > 1.0 else np.float32(0.0)
    cls = (P[4] + P[6]) / max(P[5], np.float32(1.0)) if P[2] > 1.0 else np.float32(0.0)
    return np.float32(heat + box + cls)


def kernel(pred_heatmap, pred_boxes, pred_classes, bboxes, labels):
    if "nc" not in _CACHE:
        _CACHE["nc"] = _build()
    nc = _CACHE["nc"]
    in_maps = _prep(pred_heatmap, pred_boxes, pred_classes, bboxes, labels)
    r = run_bass_kernel_spmd(nc, in_maps, list(range(NC)))
    pvecs = [np.asarray(r.results[c]["out"]).reshape(8) for c in range(NC)]
    return _combine(pvecs)


if __name__ == "__main__":
    import reference
    inputs = reference.setup_inputs()
    inputs = {k: np.asarray(v) for k, v in inputs.items()}
    out = kernel(**inputs)
    exp = np.asarray(reference.reference(**inputs))
    rel = abs(out - exp) / max(abs(exp), 1e-9)
    print("expected:", exp, "actual:", out, "rel:", rel)
